# revision 21
# baseline (speedup 1.0000x reference)
"""Trainium2 Bass kernel for a causal multi-head attention layer.

Model: b=2, s=2048, d_model=1024, 16 heads, head_dim=64, pad-index 0.
Sharding over 8 NeuronCores: each core owns 2 heads (128 of the 1024
attention dims) for both batches (head/tensor parallel).  After attention,
an AllToAll redistributes the per-head outputs so each core holds all 1024
attention dims for 1/8 of the sequence positions, where it runs the output
projection locally.  Output rows per core: 256 rows of each batch.

v2 layout/schedule:
  - scores for the two local heads run concurrently on the PE array via
    row tiling (K=64 contraction at PE rows 0-63 / 64-127).
  - PV is computed "flipped" (V-with-ones-column stationary, exp(scores)
    moving), so the output lands as [dims, queries] -- no PE transposes.
    The 65th row of the PSUM accumulator is the softmax denominator.
  - attention runs in 512-query passes (exact causal staircase);
    normalization = DVE reciprocal + K=1 broadcast matmul + DVE multiply.
  - projection matmuls for the *other* batch and the output projections
    are interleaved into the attention instruction stream as PE filler so
    the tensor engine never idles (keeps the HAM clock gate at 8/8).
  - AllToAll chunks (1024 queries) are issued as soon as their passes
    finish, overlapping the remaining attention compute.
"""

import threading

import numpy as np

B, S, D = 2, 2048, 1024
H, HD = 16, 64
NCORES = 8
LD = D // NCORES          # 128 local attention dims (2 heads)
R = B * S                 # 4096 flattened rows
RC = R // NCORES          # 512 output rows per core
RB = S // NCORES          # 256 rows per batch per core
NKT = S // 128            # 16 key tiles per batch
NCH = D // 128            # 8 contraction chunks of d_model
PASS = 512                # attention query-pass width
NPASS = S // PASS         # 4 passes per batch

_cache = {}
_lock = threading.Lock()


class _MGen:
    """Ordered list of (unit, thunk) with directed catch-up: ensure(u)
    emits every thunk tagged <= u; pump_one() emits the next one."""

    def __init__(self, items):
        self.items = list(items)
        self.i = 0

    def pump_one(self):
        if self.i < len(self.items):
            self.items[self.i][1]()
            self.i += 1
            return True
        return False

    def ensure(self, unit):
        while self.i < len(self.items) and self.items[self.i][0] <= unit:
            self.items[self.i][1]()
            self.i += 1


class _Filler:
    """FIFO of thunk sources, pumped between attention ops to keep the
    tensor engine busy during softmax-bound stretches."""

    def __init__(self):
        self.srcs = []

    def add(self, src):
        if not isinstance(src, _MGen):
            src = _MGen([(0, t) for t in src])
        self.srcs.append(src)

    def pump(self, n):
        while n > 0 and self.srcs:
            if self.srcs[0].pump_one():
                n -= 1
            else:
                self.srcs.pop(0)

    def drain(self):
        while self.srcs:
            self.pump(1)


def _build_nc():
    import concourse.mybir as mybir
    import concourse.tile as tile
    from concourse import bacc
    from contextlib import ExitStack

    f32 = mybir.dt.float32
    bf16 = mybir.dt.bfloat16
    i32 = mybir.dt.int32
    AF = mybir.ActivationFunctionType
    ALU = mybir.AluOpType

    nc = bacc.Bacc(None, target_bir_lowering=False, num_devices=NCORES)

    xT = nc.declare_dram_parameter("xT", [D, R], bf16, isOutput=False)
    wqT = nc.declare_dram_parameter("wqT", [D, LD], bf16, isOutput=False)
    wkT = nc.declare_dram_parameter("wkT", [D, LD], bf16, isOutput=False)
    wvT = nc.declare_dram_parameter("wvT", [D, LD], bf16, isOutput=False)
    woT = nc.declare_dram_parameter("woT", [D, D], bf16, isOutput=False)
    bq = nc.declare_dram_parameter("bq", [LD], f32, isOutput=False)
    bk = nc.declare_dram_parameter("bk", [LD], f32, isOutput=False)
    bv = nc.declare_dram_parameter("bv", [LD], f32, isOutput=False)
    bo = nc.declare_dram_parameter("bo", [D], f32, isOutput=False)
    ids = nc.declare_dram_parameter("ids", [128, B * NKT], i32, isOutput=False)
    out = nc.declare_dram_parameter("out", [RC, D], f32, isOutput=True)

    with ExitStack() as ctx:
        tc = ctx.enter_context(tile.TileContext(nc))
        const = ctx.enter_context(tc.tile_pool(name="const", bufs=1))
        qkp = ctx.enter_context(tc.tile_pool(name="qkp", bufs=2))
        work = ctx.enter_context(tc.tile_pool(name="work", bufs=4))
        epool = ctx.enter_context(tc.tile_pool(name="epool", bufs=3))
        stg = ctx.enter_context(tc.tile_pool(name="stg", bufs=2))
        dpool = ctx.enter_context(tc.tile_pool(name="dram", bufs=2, space="DRAM"))

        # ---- constants (small weights first so compute can start early) ----
        wqT_sb = const.tile([128, NCH, LD], bf16)
        nc.sync.dma_start(wqT_sb, wqT.ap().rearrange("(c p) d -> p c d", p=128))
        wkT_sb = const.tile([128, NCH, LD], bf16)
        nc.sync.dma_start(wkT_sb, wkT.ap().rearrange("(c p) d -> p c d", p=128))
        wvT_sb = const.tile([128, NCH, LD], bf16)
        nc.sync.dma_start(wvT_sb, wvT.ap().rearrange("(c p) d -> p c d", p=128))
        bq_col = const.tile([128, 1], f32)
        nc.sync.dma_start(bq_col, bq.ap().rearrange("(p o) -> p o", o=1))
        bk_col = const.tile([128, 1], f32)
        nc.sync.dma_start(bk_col, bk.ap().rearrange("(p o) -> p o", o=1))
        bv_bc = const.tile([128, LD], f32)
        nc.sync.dma_start(bv_bc, bv.ap().partition_broadcast(128))
        ids_sb = const.tile([128, B * NKT], i32)
        nc.sync.dma_start(ids_sb, ids.ap())

        # x^T per (batch, contraction chunk): [128, S] tiles.  Batch 0's
        # chunks first so its QK projection starts after the first lands.
        xTr = xT.ap().rearrange("(c p) (b r) -> b c p r", p=128, b=B)
        xb = [[None] * NCH for _ in range(B)]
        for b in range(B):
            for c in range(NCH):
                t = const.tile([128, S], bf16, name=f"x{b}c{c}", tag=f"x{b}c{c}")
                nc.sync.dma_start(t, xTr[b, c])
                xb[b][c] = t
        woT_sb = const.tile([128, NCH, D], bf16)
        nc.sync.dma_start(woT_sb, woT.ap().rearrange("(c p) n -> p c n", p=128))
        bo_bc = const.tile([128, D], f32)
        nc.sync.dma_start(bo_bc, bo.ap().partition_broadcast(128))

        padf = const.tile([128, B * NKT], f32)
        nc.vector.tensor_copy(padf, ids_sb)
        nc.vector.tensor_scalar_min(padf, padf, 1.0)

        ones_row = const.tile([1, HD], bf16)
        nc.gpsimd.memset(ones_row, 1.0)
        # diagmask[x, y] = 1 if y >= x else 0  (keys on partitions)
        diagmask = const.tile([128, 128], bf16)
        nc.gpsimd.memset(diagmask, 1.0)
        nc.gpsimd.affine_select(
            out=diagmask, in_=diagmask, compare_op=ALU.is_ge, fill=0.0,
            base=0, pattern=[[1, 128]], channel_multiplier=-1,
        )

        qt = [None] * B
        kt = [None] * B
        vaug = [None] * B
        stage = [None] * B

        # ---- batch 0 QK projection, contraction-outer (DMA-pipelined) ----
        # Uses 8 PSUM banks transiently; the pool closes before the
        # attention-phase PSUM pools open.
        qt[0] = qkp.tile([128, S], bf16, name="qt0", tag="qt")
        kt[0] = qkp.tile([128, S], bf16, name="kt0", tag="kt")
        with tc.tile_pool(name="qk8", bufs=1, space="PSUM") as qk8:
            pq = qk8.tile([128, S], f32, name="pq0", tag="pq")
            pk = qk8.tile([128, S], f32, name="pk0", tag="pk")
            for c in range(NCH):
                st, sp = c == 0, c == NCH - 1
                for q4 in range(S // 512):
                    sl = slice(q4 * 512, (q4 + 1) * 512)
                    nc.tensor.matmul(pq[:, sl], wqT_sb[:, c, :],
                                     xb[0][c][:, sl], start=st, stop=sp)
                    nc.tensor.matmul(pk[:, sl], wkT_sb[:, c, :],
                                     xb[0][c][:, sl], start=st, stop=sp)
            for q4 in range(S // 512):
                sl = slice(q4 * 512, (q4 + 1) * 512)
                nc.vector.tensor_scalar_add(qt[0][:, sl], pq[:, sl], bq_col)
                nc.vector.tensor_scalar_add(kt[0][:, sl], pk[:, sl], bk_col)

        # ---- attention-phase PSUM pools (exactly 8 banks) ----
        scp = ctx.enter_context(tc.tile_pool(name="scp", bufs=1, space="PSUM"))
        pvp = ctx.enter_context(tc.tile_pool(name="pvp", bufs=1, space="PSUM"))
        fil = ctx.enter_context(tc.tile_pool(name="fil", bufs=2, space="PSUM"))

        def v_proj_mgen(b):
            """V projection in [keys, dims] layout + bias/pad/ones -> vaug.
            Two thunks per key tile m (4 contraction chunks each); unit = m
            so attention passes can ensure() the tiles they need."""
            vaug[b] = qkp.tile([128, 2, NKT, HD + 1], bf16,
                               name=f"vaug{b}", tag="vaug")
            items = []
            for m in range(NKT):
                hold = [None]

                def goA(m=m, b=b, hold=hold):
                    rsl = slice(m * 128, (m + 1) * 128)
                    hold[0] = fil.tile([128, 512], f32, name="pv", tag="fil")
                    for c in range(4):
                        nc.tensor.matmul(hold[0][:, 0:LD], xb[b][c][:, rsl],
                                         wvT_sb[:, c, :],
                                         start=(c == 0), stop=False)

                def goB(m=m, b=b, hold=hold):
                    rsl = slice(m * 128, (m + 1) * 128)
                    pvt = hold[0]
                    for c in range(4, NCH):
                        nc.tensor.matmul(pvt[:, 0:LD], xb[b][c][:, rsl],
                                         wvT_sb[:, c, :],
                                         start=False, stop=(c == NCH - 1))
                    tv = work.tile([128, LD], f32, name="tv", tag="tv")
                    nc.vector.tensor_add(tv, pvt[:, 0:LD], bv_bc)
                    pcol = padf[:, b * NKT + m:b * NKT + m + 1]
                    for h in range(2):
                        nc.vector.tensor_scalar_mul(
                            vaug[b][:, h, m, 0:HD], tv[:, h * HD:(h + 1) * HD],
                            pcol)
                        nc.vector.tensor_copy(vaug[b][:, h, m, HD:HD + 1], pcol)
                items.append((m, goA))
                items.append((m, goB))
            return _MGen(items)

        def qk_proj_mgen(b):
            """QK projection as filler thunks (two per 512-row quarter per
            q/k; x for batch b must be resident when these run)."""
            qt[b] = qkp.tile([128, S], bf16, name=f"qt{b}", tag="qt")
            kt[b] = qkp.tile([128, S], bf16, name=f"kt{b}", tag="kt")
            items = []
            for q4 in range(S // 512):
                for wsb, bcol, dstl in ((wqT_sb, bq_col, qt),
                                        (wkT_sb, bk_col, kt)):
                    hold = [None]

                    def goA(q4=q4, b=b, wsb=wsb, hold=hold):
                        sl = slice(q4 * 512, (q4 + 1) * 512)
                        hold[0] = fil.tile([128, 512], f32, name="pq",
                                           tag="fil")
                        for c in range(4):
                            nc.tensor.matmul(hold[0], wsb[:, c, :],
                                             xb[b][c][:, sl],
                                             start=(c == 0), stop=False)

                    def goB(q4=q4, b=b, wsb=wsb, bcol=bcol, dstl=dstl,
                            hold=hold):
                        sl = slice(q4 * 512, (q4 + 1) * 512)
                        for c in range(4, NCH):
                            nc.tensor.matmul(hold[0], wsb[:, c, :],
                                             xb[b][c][:, sl],
                                             start=False, stop=(c == NCH - 1))
                        nc.vector.tensor_scalar_add(dstl[b][:, sl], hold[0],
                                                    bcol)
                    items.append((q4, goA))
                    items.append((q4, goB))
            return _MGen(items)

        def outproj_thunks(b, pp, a2a_out):
            """Output projection for one gathered 128-row chunk (the final
            output rows ride the sync DMA queue)."""
            a2a_sb = stg.tile([128, NCORES, 128], bf16, name=f"a2as{b}{pp}",
                              tag="a2as", bufs=4)

            def load(a2a_sb=a2a_sb, a2a_out=a2a_out):
                nc.sync.dma_start(
                    a2a_sb, a2a_out.rearrange("(j p) r -> p j r", p=128))
            yield load
            r0 = b * RB + pp * 128
            for n in range(D // 512):
                hold = [None]

                def goA(n=n, a2a_sb=a2a_sb, hold=hold):
                    hold[0] = fil.tile([128, 512], f32, name="po", tag="fil")
                    for c in range(4):
                        nc.tensor.matmul(
                            hold[0], a2a_sb[:, c, :],
                            woT_sb[:, c, n * 512:(n + 1) * 512],
                            start=(c == 0), stop=False)
                yield goA

                def goB(n=n, a2a_sb=a2a_sb, r0=r0, hold=hold):
                    for c in range(4, NCH):
                        nc.tensor.matmul(
                            hold[0], a2a_sb[:, c, :],
                            woT_sb[:, c, n * 512:(n + 1) * 512],
                            start=False, stop=(c == NCH - 1))
                    ot = work.tile([128, 512], f32, name="ot", tag="ot")
                    nc.vector.tensor_add(ot, hold[0],
                                         bo_bc[:, n * 512:(n + 1) * 512])
                    nc.sync.dma_start(
                        out.ap()[r0:r0 + 128, n * 512:(n + 1) * 512], ot)
                yield goB

        a2a_filler = [None]

        def outproj_half_thunks(b, p, a2a_out):
            """Output projection for one gathered 64-row (half) chunk."""
            a2a_sb = stg.tile([128, NCORES, 64], bf16, name=f"a2ah{b}{p}",
                              tag="a2ah", bufs=2)

            def load(a2a_sb=a2a_sb, a2a_out=a2a_out):
                nc.sync.dma_start(
                    a2a_sb, a2a_out.rearrange("(j p) r -> p j r", p=128))
            yield load
            r0 = b * RB + (p // 2) * 128 + (p % 2) * 64
            for n in range(D // 512):
                def go(n=n, a2a_sb=a2a_sb, r0=r0):
                    pout = fil.tile([128, 512], f32, name="po", tag="fil")
                    for c in range(NCH):
                        nc.tensor.matmul(
                            pout[0:HD, :], a2a_sb[:, c, :],
                            woT_sb[:, c, n * 512:(n + 1) * 512],
                            start=(c == 0), stop=(c == NCH - 1))
                    ot = work.tile([HD, 512], f32, name="oth", tag="oth")
                    nc.vector.tensor_add(ot, pout[0:HD, :],
                                         bo_bc[0:HD, n * 512:(n + 1) * 512])
                    nc.sync.dma_start(
                        out.ap()[r0:r0 + HD, n * 512:(n + 1) * 512], ot)
                yield go

        def issue_a2a_half(b, p, stage_b):
            """AllToAll a single 512-query pass (64 queries per core) so the
            final chunk's latency chain is as short as possible."""
            a2a_in = dpool.tile([NCORES * 128, 64], bf16,
                                name=f"a2aih{b}{p}", tag="a2aih", bufs=2)
            nc.sync.dma_start(
                a2a_in.rearrange("(j p) r -> p j r", p=128),
                stage_b[:, p * PASS:(p + 1) * PASS]
                .rearrange("p (j r) -> p j r", j=NCORES))
            a2a_out = dpool.tile([NCORES * 128, 64], bf16,
                                 name=f"a2aoh{b}{p}", tag="a2aoh", bufs=2)
            nc.gpsimd.collective_compute(
                "AllToAll", ALU.bypass,
                replica_groups=[list(range(NCORES))],
                ins=[a2a_in.opt()], outs=[a2a_out.opt()])
            a2a_filler[0].add(outproj_half_thunks(b, p, a2a_out))

        def issue_a2a(b, pp):
            """AllToAll one 1024-query chunk of batch b's stage buffer; its
            output projection becomes filler work."""
            a2a_in = dpool.tile([NCORES * 128, 128], bf16,
                                name=f"a2ai{b}{pp}", tag="a2ai", bufs=4)
            nc.sync.dma_start(
                a2a_in.rearrange("(j p) r -> p j r", p=128),
                stage[b][:, pp * 1024:(pp + 1) * 1024]
                .rearrange("p (j r) -> p j r", j=NCORES))
            a2a_out = dpool.tile([NCORES * 128, 128], bf16,
                                 name=f"a2ao{b}{pp}", tag="a2ao", bufs=4)
            nc.gpsimd.collective_compute(
                "AllToAll", ALU.bypass,
                replica_groups=[list(range(NCORES))],
                ins=[a2a_in.opt()], outs=[a2a_out.opt()])
            a2a_filler[0].add(outproj_thunks(b, pp, a2a_out))

        def attention(b, filler, vgen, fast_tail=False):
            """Attention for batch b, both heads, 512-query passes.  PV is
            pipelined one score-pair behind exp.  Normalization for a
            1024-query chunk is deferred into the next pass so the PE queue
            never waits on the reciprocal chain."""
            stage[b] = stg.tile([128, S], bf16, name=f"stage{b}", tag="stage")
            pvsb = {}            # (h, pp) -> [65, 1024] f32 SBUF copy of PV
            pending = []         # deferred normalization closures

            def norm_half(p):
                """Per-pass DMA-chain normalization + half A2A (used for the
                next-to-last pass of the fast tail)."""
                pp = p // 2
                csl = slice((p % 2) * PASS, (p % 2 + 1) * PASS)
                qsl = slice(p * PASS, (p + 1) * PASS)
                for h in range(2):
                    ps = pvsb[(h, pp)]
                    dr_den = dpool.tile([PASS], f32, name="drdh",
                                        tag="drdh", bufs=2)
                    nc.sync.dma_start(dr_den, ps[HD:HD + 1, csl])
                    denT = work.tile([128, 4], f32, name="denTh", tag="denTh")
                    nc.sync.dma_start(
                        denT, dr_den.rearrange("(p f) -> p f", p=128))
                    rcpT = work.tile([128, 4], bf16, name="rcpTh", tag="rcpTh")
                    with nc.allow_low_precision(reason="softmax denom bf16"):
                        nc.vector.reciprocal(rcpT, denT)
                    dr_rec = dpool.tile([PASS], bf16, name="drrh",
                                        tag="drrh", bufs=2)
                    nc.sync.dma_start(
                        dr_rec.rearrange("(p f) -> p f", p=128), rcpT)
                    bcs = work.tile([HD, PASS], bf16, name="bcsh", tag="bcsh")
                    nc.sync.dma_start(bcs, dr_rec.partition_broadcast(HD))
                    nc.vector.tensor_mul(
                        stage[b][h * HD:(h + 1) * HD, qsl], ps[0:HD, csl], bcs)
                issue_a2a_half(b, p, stage[b])

            def norm_fast(p):
                """Latency-optimal normalization for the very last pass:
                direct DVE reciprocal + ones-matmul broadcast (no DMA round
                trips), then the half A2A."""
                pp = p // 2
                csl = slice((p % 2) * PASS, (p % 2 + 1) * PASS)
                qsl = slice(p * PASS, (p + 1) * PASS)
                for h in range(2):
                    ps = pvsb[(h, pp)]
                    rec = work.tile([1, PASS], bf16, name="recf", tag="recf")
                    with nc.allow_low_precision(reason="softmax denom bf16"):
                        nc.vector.reciprocal(rec, ps[HD:HD + 1, csl])
                    bc = fil.tile([HD, PASS], f32, name="bcf", tag="fil")
                    nc.tensor.matmul(bc, ones_row, rec, start=True, stop=True)
                    nc.vector.tensor_mul(
                        stage[b][h * HD:(h + 1) * HD, qsl], ps[0:HD, csl], bc)
                issue_a2a_half(b, p, stage[b])

            def norm_pp(pp):
                """Normalize one 1024-query chunk of both heads into the
                staging buffer.  The denominator row is round-tripped
                through DRAM so the reciprocal runs at free-size 8 (instead
                of [1, 1024], where DVE reciprocal costs ~6.4 cyc/elem) and
                the reciprocal is broadcast across partitions by the DMA
                reload; then the chunk's AllToAll is issued."""
                qsl = slice(pp * 1024, (pp + 1) * 1024)
                for h in range(2):
                    ps = pvsb[(h, pp)]
                    dr_den = dpool.tile([2 * PASS], f32, name="drden",
                                        tag="drden", bufs=2)
                    nc.sync.dma_start(dr_den, ps[HD:HD + 1, :])
                    denT = work.tile([128, 8], f32, name="denT", tag="denT")
                    nc.sync.dma_start(
                        denT, dr_den.rearrange("(p f) -> p f", p=128))
                    rcpT = work.tile([128, 8], bf16, name="rcpT", tag="rcpT")
                    with nc.allow_low_precision(reason="softmax denom bf16"):
                        nc.vector.reciprocal(rcpT, denT)
                    dr_rec = dpool.tile([2 * PASS], bf16, name="drrec",
                                        tag="drrec", bufs=2)
                    nc.sync.dma_start(
                        dr_rec.rearrange("(p f) -> p f", p=128), rcpT)
                    bcs = work.tile([HD, 2 * PASS], bf16, name="bcs",
                                    tag="bcs")
                    nc.sync.dma_start(bcs, dr_rec.partition_broadcast(HD))
                    nc.vector.tensor_mul(
                        stage[b][h * HD:(h + 1) * HD, qsl], ps[0:HD, :], bcs)
                issue_a2a(b, pp)

            def run_pending():
                while pending:
                    pending.pop(0)()

            for p in range(NPASS):
                vgen.ensure(4 * p + 3)
                q0 = p * PASS
                ks = list(range(4 * p + 4))      # visible key tiles
                pairs = []
                for i in range(0, len(ks), 2):
                    grp = ks[i:i + 2]
                    pairs.append([(k2, min(PASS, q0 + PASS - 128 * k2))
                                  for k2 in grp])
                npair = len(pairs)
                pv0 = pvp.tile([HD + 1, PASS], f32, name="pv0", tag="pv0")
                pv1 = pvp.tile([HD + 1, PASS], f32, name="pv1", tag="pv1")
                etiles = [None] * npair

                def do_pv(pi):
                    ep, widths = etiles[pi]
                    off = 0
                    for j, (k2, w) in enumerate(widths):
                        st = pi == 0 and j == 0
                        sp = pi == npair - 1 and j == len(widths) - 1
                        psl = slice(PASS - w, PASS)
                        nc.tensor.matmul(pv0[:, psl], vaug[b][:, 0, k2, :],
                                         ep[:, off:off + w],
                                         start=st, stop=sp)
                        nc.tensor.matmul(pv1[:, psl], vaug[b][:, 1, k2, :],
                                         ep[:, 1024 + off:1024 + off + w],
                                         start=st, stop=sp)
                        off += w

                for pi, widths in enumerate(pairs):
                    # scores for this pair: h0 -> columns [0, 1024), h1 ->
                    # [1024, 2048) of one 4-bank PSUM tile (row-tiled MMs of
                    # the two heads run concurrently on the PE array)
                    sp = scp.tile([128, 2048], f32, name="s", tag="s")
                    off = 0
                    for k2, w in widths:
                        kA = slice(k2 * 128, (k2 + 1) * 128)
                        qA = slice(q0 + PASS - w, q0 + PASS)
                        nc.tensor.matmul(sp[:, off:off + w],
                                         kt[b][0:HD, kA], qt[b][0:HD, qA],
                                         start=True, stop=True)
                        nc.tensor.matmul(sp[:, 1024 + off:1024 + off + w],
                                         kt[b][HD:128, kA], qt[b][HD:128, qA],
                                         start=True, stop=True)
                        off += w
                    if pi == min(2, npair - 1):
                        run_pending()
                    filler.pump(3)
                    # exp: ONE ACT instruction covers both heads' chunks
                    ep = epool.tile([128, 2048], bf16, name="e", tag="e")
                    etiles[pi] = (ep, widths)
                    if off == 1024:
                        nc.scalar.activation(ep, sp, AF.Exp, scale=0.125)
                    else:
                        nc.scalar.activation(ep[:, 0:off], sp[:, 0:off],
                                             AF.Exp, scale=0.125)
                        nc.scalar.activation(ep[:, 1024:1024 + off],
                                             sp[:, 1024:1024 + off],
                                             AF.Exp, scale=0.125)
                    # causal mask on diagonal-starting chunks
                    off = 0
                    for k2, w in widths:
                        if 128 * k2 >= q0:
                            for ho in (0, 1024):
                                nc.vector.tensor_mul(
                                    ep[:, ho + off:ho + off + 128],
                                    ep[:, ho + off:ho + off + 128], diagmask)
                        off += w
                    # PV one pair behind (exp of pair pi still in flight)
                    if pi >= 1:
                        do_pv(pi - 1)
                        filler.pump(1)
                do_pv(npair - 1)
                # free the PV accumulator banks promptly: copy to SBUF, then
                # defer normalization into a later instruction stream
                pp = p // 2
                csl = slice((p % 2) * PASS, (p % 2 + 1) * PASS)
                for h, pv in ((0, pv0), (1, pv1)):
                    if (h, pp) not in pvsb:
                        pvsb[(h, pp)] = qkp.tile(
                            [HD + 1, 2 * PASS], f32, name=f"ps{h}", tag=f"ps{h}")
                    nc.vector.tensor_copy(pvsb[(h, pp)][:, csl], pv)
                if fast_tail and p == NPASS - 2:
                    pending.append(lambda p=p: norm_half(p))
                elif fast_tail and p == NPASS - 1:
                    pass
                elif p % 2 == 1:
                    pending.append(lambda pp=pp: norm_pp(pp))
                filler.pump(2)
                if p == NPASS - 1:
                    run_pending()
                    if fast_tail:
                        filler.pump(6)
                        norm_fast(p)
                    else:
                        filler.pump(3)

        # ---------------- schedule ----------------
        # batch 0's V-projection tail and batch 1's projections fill PE
        # gaps during batch-0 attention; output projections (queued by
        # issue_a2a) and batch 1's V tail fill batch-1 attention.
        vgen0 = v_proj_mgen(0)
        vgen1 = v_proj_mgen(1)
        filler0 = _Filler()
        filler1 = _Filler()
        filler0.add(vgen0)
        filler0.add(qk_proj_mgen(1))
        filler1.add(vgen1)
        a2a_filler[0] = filler1
        attention(0, filler0, vgen0)
        filler0.drain()
        attention(1, filler1, vgen1, fast_tail=True)
        filler1.drain()

    nc.finalize()
    return nc


def _get_nc():
    with _lock:
        if "nc" not in _cache:
            _cache["nc"] = _build_nc()
        return _cache["nc"]


def _shard_inputs(x, input_ids, Wq, bq, Wk, bk, Wv, bv, Wo, bo):
    import ml_dtypes
    bf16 = ml_dtypes.bfloat16

    x = np.asarray(x, dtype=np.float32)
    xT = np.ascontiguousarray(x.reshape(R, D).T).astype(bf16)
    woT = np.ascontiguousarray(np.asarray(Wo, dtype=np.float32).T).astype(bf16)
    bo_f = np.asarray(bo, dtype=np.float32)
    ids = np.asarray(input_ids).astype(np.int32)
    # ids_r[p, b*NKT + t] = input_ids[b, t*128 + p]
    ids_r = np.ascontiguousarray(ids.reshape(B, NKT, 128).transpose(2, 0, 1)
                                 .reshape(128, B * NKT))
    Wq = np.asarray(Wq, dtype=np.float32)
    Wk = np.asarray(Wk, dtype=np.float32)
    Wv = np.asarray(Wv, dtype=np.float32)
    bq = np.asarray(bq, dtype=np.float32)
    bk = np.asarray(bk, dtype=np.float32)
    bv = np.asarray(bv, dtype=np.float32)

    in_maps = []
    for c in range(NCORES):
        sl = slice(c * LD, (c + 1) * LD)
        in_maps.append({
            "xT": xT,
            "wqT": np.ascontiguousarray(Wq[sl].T).astype(bf16),
            "wkT": np.ascontiguousarray(Wk[sl].T).astype(bf16),
            "wvT": np.ascontiguousarray(Wv[sl].T).astype(bf16),
            "woT": woT,
            "bq": bq[sl].copy(),
            "bk": bk[sl].copy(),
            "bv": bv[sl].copy(),
            "bo": bo_f,
            "ids": ids_r,
        })
    return in_maps


def run(trace=False, **inputs):
    """Run the kernel; returns (output, BassKernelResults)."""
    from concourse.bass_utils import run_bass_kernel_spmd

    nc = _get_nc()
    in_maps = _shard_inputs(**inputs)
    res = run_bass_kernel_spmd(nc, in_maps, core_ids=list(range(NCORES)),
                               trace=trace)
    full = np.empty((B, S, D), dtype=np.float32)
    for c in range(NCORES):
        o = np.asarray(res.results[c]["out"], dtype=np.float32)
        for b in range(B):
            for t in range(2):
                if b == 1 and t == 1:
                    continue
                full[b, t * 1024 + c * 128:t * 1024 + (c + 1) * 128, :] = \
                    o[b * RB + t * 128:b * RB + (t + 1) * 128, :]
        # batch 1's last 1024 queries travel as two per-pass half A2As:
        # core c holds queries p*512 + c*64 of passes p = 2, 3
        full[1, 1024 + c * 64:1024 + (c + 1) * 64, :] = o[384:448, :]
        full[1, 1536 + c * 64:1536 + (c + 1) * 64, :] = o[448:512, :]
    return full, res


def kernel(**inputs) -> np.ndarray:
    full, _ = run(trace=False, **inputs)
    return full


# revision 23
# speedup vs baseline: 1.0308x; 1.0308x over previous
"""Trainium2 Bass kernel for a causal multi-head attention layer.

Model: b=2, s=2048, d_model=1024, 16 heads, head_dim=64, pad-index 0.
Sharding over 8 NeuronCores: each core owns 2 heads (128 of the 1024
attention dims) for both batches (head/tensor parallel).  After attention,
an AllToAll redistributes the per-head outputs so each core holds all 1024
attention dims for 1/8 of the sequence positions, where it runs the output
projection locally.  Output rows per core: 256 rows of each batch.

v2 layout/schedule:
  - scores for the two local heads run concurrently on the PE array via
    row tiling (K=64 contraction at PE rows 0-63 / 64-127).
  - PV is computed "flipped" (V-with-ones-column stationary, exp(scores)
    moving), so the output lands as [dims, queries] -- no PE transposes.
    The 65th row of the PSUM accumulator is the softmax denominator.
  - attention runs in 512-query passes (exact causal staircase);
    normalization = DVE reciprocal + K=1 broadcast matmul + DVE multiply.
  - projection matmuls for the *other* batch and the output projections
    are interleaved into the attention instruction stream as PE filler so
    the tensor engine never idles (keeps the HAM clock gate at 8/8).
  - AllToAll chunks (1024 queries) are issued as soon as their passes
    finish, overlapping the remaining attention compute.
"""

import threading

import numpy as np

B, S, D = 2, 2048, 1024
H, HD = 16, 64
NCORES = 8
LD = D // NCORES          # 128 local attention dims (2 heads)
R = B * S                 # 4096 flattened rows
RC = R // NCORES          # 512 output rows per core
RB = S // NCORES          # 256 rows per batch per core
NKT = S // 128            # 16 key tiles per batch
NCH = D // 128            # 8 contraction chunks of d_model
PASS = 512                # attention query-pass width
NPASS = S // PASS         # 4 passes per batch

_cache = {}
_lock = threading.Lock()


class _MGen:
    """Ordered list of (unit, thunk) with directed catch-up: ensure(u)
    emits every thunk tagged <= u; pump_one() emits the next one."""

    def __init__(self, items):
        self.items = list(items)
        self.i = 0

    def pump_one(self):
        if self.i < len(self.items):
            self.items[self.i][1]()
            self.i += 1
            return True
        return False

    def ensure(self, unit):
        while self.i < len(self.items) and self.items[self.i][0] <= unit:
            self.items[self.i][1]()
            self.i += 1


class _Filler:
    """FIFO of thunk sources, pumped between attention ops to keep the
    tensor engine busy during softmax-bound stretches.  A source added
    with delay=k stays locked for the next k pump ticks, so thunks whose
    data dependencies are still in flight (e.g. an output projection
    whose AllToAll hasn't landed) don't block the in-order PE queue."""

    def __init__(self):
        self.srcs = []
        self.tick = 0

    def add(self, src, delay=0):
        if not isinstance(src, _MGen):
            src = _MGen([(0, t) for t in src])
        self.srcs.append((self.tick + delay, src))

    def pump(self, n):
        while n > 0:
            self.tick += 1
            n -= 1
            for j, (ready_at, src) in enumerate(self.srcs):
                if ready_at > self.tick:
                    continue
                if src.pump_one():
                    break
                else:
                    self.srcs.pop(j)
                    break
            else:
                return

    def drain(self):
        while self.srcs:
            changed = False
            for ready_at, src in list(self.srcs):
                if src.pump_one():
                    changed = True
                else:
                    self.srcs = [(r, s) for r, s in self.srcs if s is not src]
            if not changed and not self.srcs:
                break


def _build_nc():
    import concourse.mybir as mybir
    import concourse.tile as tile
    from concourse import bacc
    from contextlib import ExitStack

    f32 = mybir.dt.float32
    bf16 = mybir.dt.bfloat16
    i32 = mybir.dt.int32
    AF = mybir.ActivationFunctionType
    ALU = mybir.AluOpType

    nc = bacc.Bacc(None, target_bir_lowering=False, num_devices=NCORES)

    xT = nc.declare_dram_parameter("xT", [D, R], bf16, isOutput=False)
    wqT = nc.declare_dram_parameter("wqT", [D, LD], bf16, isOutput=False)
    wkT = nc.declare_dram_parameter("wkT", [D, LD], bf16, isOutput=False)
    wvT = nc.declare_dram_parameter("wvT", [D, LD], bf16, isOutput=False)
    woT = nc.declare_dram_parameter("woT", [D, D], bf16, isOutput=False)
    bq = nc.declare_dram_parameter("bq", [LD], f32, isOutput=False)
    bk = nc.declare_dram_parameter("bk", [LD], f32, isOutput=False)
    bv = nc.declare_dram_parameter("bv", [LD], f32, isOutput=False)
    bo = nc.declare_dram_parameter("bo", [D], f32, isOutput=False)
    ids = nc.declare_dram_parameter("ids", [128, B * NKT], i32, isOutput=False)
    out = nc.declare_dram_parameter("out", [RC, D], f32, isOutput=True)

    with ExitStack() as ctx:
        tc = ctx.enter_context(tile.TileContext(nc))
        const = ctx.enter_context(tc.tile_pool(name="const", bufs=1))
        qkp = ctx.enter_context(tc.tile_pool(name="qkp", bufs=2))
        work = ctx.enter_context(tc.tile_pool(name="work", bufs=4))
        epool = ctx.enter_context(tc.tile_pool(name="epool", bufs=3))
        stg = ctx.enter_context(tc.tile_pool(name="stg", bufs=2))
        dpool = ctx.enter_context(tc.tile_pool(name="dram", bufs=2, space="DRAM"))

        # ---- constants (small weights first so compute can start early) ----
        wqT_sb = const.tile([128, NCH, LD], bf16)
        nc.sync.dma_start(wqT_sb, wqT.ap().rearrange("(c p) d -> p c d", p=128))
        wkT_sb = const.tile([128, NCH, LD], bf16)
        nc.sync.dma_start(wkT_sb, wkT.ap().rearrange("(c p) d -> p c d", p=128))
        wvT_sb = const.tile([128, NCH, LD], bf16)
        nc.sync.dma_start(wvT_sb, wvT.ap().rearrange("(c p) d -> p c d", p=128))
        bq_col = const.tile([128, 1], f32)
        nc.sync.dma_start(bq_col, bq.ap().rearrange("(p o) -> p o", o=1))
        bk_col = const.tile([128, 1], f32)
        nc.sync.dma_start(bk_col, bk.ap().rearrange("(p o) -> p o", o=1))
        bv_bc = const.tile([128, LD], f32)
        nc.sync.dma_start(bv_bc, bv.ap().partition_broadcast(128))
        ids_sb = const.tile([128, B * NKT], i32)
        nc.sync.dma_start(ids_sb, ids.ap())

        # x^T per (batch, contraction chunk): [128, S] tiles.  Batch 0's
        # chunks first so its QK projection starts after the first lands.
        xTr = xT.ap().rearrange("(c p) (b r) -> b c p r", p=128, b=B)
        xb = [[None] * NCH for _ in range(B)]
        for b in range(B):
            for c in range(NCH):
                t = const.tile([128, S], bf16, name=f"x{b}c{c}", tag=f"x{b}c{c}")
                nc.sync.dma_start(t, xTr[b, c])
                xb[b][c] = t
        woT_sb = const.tile([128, NCH, D], bf16)
        nc.sync.dma_start(woT_sb, woT.ap().rearrange("(c p) n -> p c n", p=128))
        bo_bc = const.tile([128, D], f32)
        nc.sync.dma_start(bo_bc, bo.ap().partition_broadcast(128))

        padf = const.tile([128, B * NKT], f32)
        nc.vector.tensor_copy(padf, ids_sb)
        nc.vector.tensor_scalar_min(padf, padf, 1.0)

        ones_row = const.tile([1, HD], bf16)
        nc.gpsimd.memset(ones_row, 1.0)
        # diagmask[x, y] = 1 if y >= x else 0  (keys on partitions)
        diagmask = const.tile([128, 128], bf16)
        nc.gpsimd.memset(diagmask, 1.0)
        nc.gpsimd.affine_select(
            out=diagmask, in_=diagmask, compare_op=ALU.is_ge, fill=0.0,
            base=0, pattern=[[1, 128]], channel_multiplier=-1,
        )

        qt = [None] * B
        kt = [None] * B
        vaug = [None] * B
        stage = [None] * B

        # ---- batch 0 QK projection, contraction-outer (DMA-pipelined) ----
        # Uses 8 PSUM banks transiently; the pool closes before the
        # attention-phase PSUM pools open.
        qt[0] = qkp.tile([128, S], bf16, name="qt0", tag="qt")
        kt[0] = qkp.tile([128, S], bf16, name="kt0", tag="kt")
        with tc.tile_pool(name="qk8", bufs=1, space="PSUM") as qk8:
            pq = qk8.tile([128, S], f32, name="pq0", tag="pq")
            pk = qk8.tile([128, S], f32, name="pk0", tag="pk")
            for c in range(NCH):
                st, sp = c == 0, c == NCH - 1
                for q4 in range(S // 512):
                    sl = slice(q4 * 512, (q4 + 1) * 512)
                    nc.tensor.matmul(pq[:, sl], wqT_sb[:, c, :],
                                     xb[0][c][:, sl], start=st, stop=sp)
                    nc.tensor.matmul(pk[:, sl], wkT_sb[:, c, :],
                                     xb[0][c][:, sl], start=st, stop=sp)
            for q4 in range(S // 512):
                sl = slice(q4 * 512, (q4 + 1) * 512)
                nc.vector.tensor_scalar_add(qt[0][:, sl], pq[:, sl], bq_col)
                nc.vector.tensor_scalar_add(kt[0][:, sl], pk[:, sl], bk_col)

        # ---- attention-phase PSUM pools (exactly 8 banks) ----
        scp = ctx.enter_context(tc.tile_pool(name="scp", bufs=1, space="PSUM"))
        pvp = ctx.enter_context(tc.tile_pool(name="pvp", bufs=1, space="PSUM"))
        fil = ctx.enter_context(tc.tile_pool(name="fil", bufs=2, space="PSUM"))

        def v_proj_mgen(b):
            """V projection in [keys, dims] layout + bias/pad/ones -> vaug.
            Two thunks per key tile m (4 contraction chunks each); unit = m
            so attention passes can ensure() the tiles they need."""
            vaug[b] = qkp.tile([128, 2, NKT, HD + 1], bf16,
                               name=f"vaug{b}", tag="vaug")
            items = []
            for m in range(NKT):
                hold = [None]

                def goA(m=m, b=b, hold=hold):
                    rsl = slice(m * 128, (m + 1) * 128)
                    hold[0] = fil.tile([128, 512], f32, name="pv", tag="fil")
                    for c in range(4):
                        nc.tensor.matmul(hold[0][:, 0:LD], xb[b][c][:, rsl],
                                         wvT_sb[:, c, :],
                                         start=(c == 0), stop=False)

                def goB(m=m, b=b, hold=hold):
                    rsl = slice(m * 128, (m + 1) * 128)
                    pvt = hold[0]
                    for c in range(4, NCH):
                        nc.tensor.matmul(pvt[:, 0:LD], xb[b][c][:, rsl],
                                         wvT_sb[:, c, :],
                                         start=False, stop=(c == NCH - 1))
                    tv = work.tile([128, LD], f32, name="tv", tag="tv")
                    nc.vector.tensor_add(tv, pvt[:, 0:LD], bv_bc)
                    pcol = padf[:, b * NKT + m:b * NKT + m + 1]
                    for h in range(2):
                        nc.vector.tensor_scalar_mul(
                            vaug[b][:, h, m, 0:HD], tv[:, h * HD:(h + 1) * HD],
                            pcol)
                        nc.vector.tensor_copy(vaug[b][:, h, m, HD:HD + 1], pcol)
                items.append((m, goA))
                items.append((m, goB))
            return _MGen(items)

        def qk_proj_mgen(b):
            """QK projection as filler thunks (two per 512-row quarter per
            q/k; x for batch b must be resident when these run)."""
            qt[b] = qkp.tile([128, S], bf16, name=f"qt{b}", tag="qt")
            kt[b] = qkp.tile([128, S], bf16, name=f"kt{b}", tag="kt")
            items = []
            for q4 in range(S // 512):
                for wsb, bcol, dstl in ((wqT_sb, bq_col, qt),
                                        (wkT_sb, bk_col, kt)):
                    hold = [None]

                    def goA(q4=q4, b=b, wsb=wsb, hold=hold):
                        sl = slice(q4 * 512, (q4 + 1) * 512)
                        hold[0] = fil.tile([128, 512], f32, name="pq",
                                           tag="fil")
                        for c in range(4):
                            nc.tensor.matmul(hold[0], wsb[:, c, :],
                                             xb[b][c][:, sl],
                                             start=(c == 0), stop=False)

                    def goB(q4=q4, b=b, wsb=wsb, bcol=bcol, dstl=dstl,
                            hold=hold):
                        sl = slice(q4 * 512, (q4 + 1) * 512)
                        for c in range(4, NCH):
                            nc.tensor.matmul(hold[0], wsb[:, c, :],
                                             xb[b][c][:, sl],
                                             start=False, stop=(c == NCH - 1))
                        nc.vector.tensor_scalar_add(dstl[b][:, sl], hold[0],
                                                    bcol)
                    items.append((q4, goA))
                    items.append((q4, goB))
            return _MGen(items)

        def outproj_thunks(b, pp, a2a_out):
            """Output projection for one gathered 128-row chunk (the final
            output rows ride the sync DMA queue)."""
            a2a_sb = stg.tile([128, NCORES, 128], bf16, name=f"a2as{b}{pp}",
                              tag="a2as", bufs=4)

            def load(a2a_sb=a2a_sb, a2a_out=a2a_out):
                nc.sync.dma_start(
                    a2a_sb, a2a_out.rearrange("(j p) r -> p j r", p=128))
            yield load
            r0 = b * RB + pp * 128
            for n in range(D // 512):
                hold = [None]

                def goA(n=n, a2a_sb=a2a_sb, hold=hold):
                    hold[0] = fil.tile([128, 512], f32, name="po", tag="fil")
                    for c in range(4):
                        nc.tensor.matmul(
                            hold[0], a2a_sb[:, c, :],
                            woT_sb[:, c, n * 512:(n + 1) * 512],
                            start=(c == 0), stop=False)
                yield goA

                def goB(n=n, a2a_sb=a2a_sb, r0=r0, hold=hold):
                    for c in range(4, NCH):
                        nc.tensor.matmul(
                            hold[0], a2a_sb[:, c, :],
                            woT_sb[:, c, n * 512:(n + 1) * 512],
                            start=False, stop=(c == NCH - 1))
                    ot = work.tile([128, 512], f32, name="ot", tag="ot")
                    nc.vector.tensor_add(ot, hold[0],
                                         bo_bc[:, n * 512:(n + 1) * 512])
                    nc.sync.dma_start(
                        out.ap()[r0:r0 + 128, n * 512:(n + 1) * 512], ot)
                yield goB

        a2a_filler = [None]

        def outproj_half_thunks(b, p, a2a_out):
            """Output projection for one gathered 64-row (half) chunk."""
            a2a_sb = stg.tile([128, NCORES, 64], bf16, name=f"a2ah{b}{p}",
                              tag="a2ah", bufs=2)

            def load(a2a_sb=a2a_sb, a2a_out=a2a_out):
                nc.sync.dma_start(
                    a2a_sb, a2a_out.rearrange("(j p) r -> p j r", p=128))
            yield load
            r0 = b * RB + (p // 2) * 128 + (p % 2) * 64
            for n in range(D // 512):
                def go(n=n, a2a_sb=a2a_sb, r0=r0):
                    pout = fil.tile([128, 512], f32, name="po", tag="fil")
                    for c in range(NCH):
                        nc.tensor.matmul(
                            pout[0:HD, :], a2a_sb[:, c, :],
                            woT_sb[:, c, n * 512:(n + 1) * 512],
                            start=(c == 0), stop=(c == NCH - 1))
                    ot = work.tile([HD, 512], f32, name="oth", tag="oth")
                    nc.vector.tensor_add(ot, pout[0:HD, :],
                                         bo_bc[0:HD, n * 512:(n + 1) * 512])
                    nc.sync.dma_start(
                        out.ap()[r0:r0 + HD, n * 512:(n + 1) * 512], ot)
                yield go

        def issue_a2a_half(b, p, stage_b):
            """AllToAll a single 512-query pass (64 queries per core) so the
            final chunk's latency chain is as short as possible."""
            a2a_in = dpool.tile([NCORES * 128, 64], bf16,
                                name=f"a2aih{b}{p}", tag="a2aih", bufs=2)
            nc.sync.dma_start(
                a2a_in.rearrange("(j p) r -> p j r", p=128),
                stage_b[:, p * PASS:(p + 1) * PASS]
                .rearrange("p (j r) -> p j r", j=NCORES))
            a2a_out = dpool.tile([NCORES * 128, 64], bf16,
                                 name=f"a2aoh{b}{p}", tag="a2aoh", bufs=2)
            nc.gpsimd.collective_compute(
                "AllToAll", ALU.bypass,
                replica_groups=[list(range(NCORES))],
                ins=[a2a_in.opt()], outs=[a2a_out.opt()])
            a2a_filler[0].add(outproj_half_thunks(b, p, a2a_out), delay=14)

        def issue_a2a(b, pp):
            """AllToAll one 1024-query chunk of batch b's stage buffer; its
            output projection becomes filler work."""
            a2a_in = dpool.tile([NCORES * 128, 128], bf16,
                                name=f"a2ai{b}{pp}", tag="a2ai", bufs=4)
            nc.sync.dma_start(
                a2a_in.rearrange("(j p) r -> p j r", p=128),
                stage[b][:, pp * 1024:(pp + 1) * 1024]
                .rearrange("p (j r) -> p j r", j=NCORES))
            a2a_out = dpool.tile([NCORES * 128, 128], bf16,
                                 name=f"a2ao{b}{pp}", tag="a2ao", bufs=4)
            nc.gpsimd.collective_compute(
                "AllToAll", ALU.bypass,
                replica_groups=[list(range(NCORES))],
                ins=[a2a_in.opt()], outs=[a2a_out.opt()])
            a2a_filler[0].add(outproj_thunks(b, pp, a2a_out), delay=14)

        def attention(b, filler, vgen, fast_tail=False):
            """Attention for batch b, both heads, 512-query passes.  PV is
            pipelined one score-pair behind exp.  Normalization for a
            1024-query chunk is deferred into the next pass so the PE queue
            never waits on the reciprocal chain."""
            stage[b] = stg.tile([128, S], bf16, name=f"stage{b}", tag="stage")
            pvsb = {}            # (h, pp) -> [65, 1024] f32 SBUF copy of PV
            pending = []         # deferred normalization closures

            def norm_half(p):
                """Per-pass DMA-chain normalization + half A2A (used for the
                next-to-last pass of the fast tail)."""
                pp = p // 2
                csl = slice((p % 2) * PASS, (p % 2 + 1) * PASS)
                qsl = slice(p * PASS, (p + 1) * PASS)
                for h in range(2):
                    ps = pvsb[(h, pp)]
                    dr_den = dpool.tile([PASS], f32, name="drdh",
                                        tag="drdh", bufs=2)
                    nc.sync.dma_start(dr_den, ps[HD:HD + 1, csl])
                    denT = work.tile([128, 4], f32, name="denTh", tag="denTh")
                    nc.sync.dma_start(
                        denT, dr_den.rearrange("(p f) -> p f", p=128))
                    rcpT = work.tile([128, 4], bf16, name="rcpTh", tag="rcpTh")
                    with nc.allow_low_precision(reason="softmax denom bf16"):
                        nc.vector.reciprocal(rcpT, denT)
                    dr_rec = dpool.tile([PASS], bf16, name="drrh",
                                        tag="drrh", bufs=2)
                    nc.sync.dma_start(
                        dr_rec.rearrange("(p f) -> p f", p=128), rcpT)
                    bcs = work.tile([HD, PASS], bf16, name="bcsh", tag="bcsh")
                    nc.sync.dma_start(bcs, dr_rec.partition_broadcast(HD))
                    nc.vector.tensor_mul(
                        stage[b][h * HD:(h + 1) * HD, qsl], ps[0:HD, csl], bcs)
                issue_a2a_half(b, p, stage[b])

            def norm_fast(p):
                """Latency-optimal normalization for the very last pass:
                direct DVE reciprocal + ones-matmul broadcast (no DMA round
                trips), then the half A2A."""
                pp = p // 2
                csl = slice((p % 2) * PASS, (p % 2 + 1) * PASS)
                qsl = slice(p * PASS, (p + 1) * PASS)
                for h in range(2):
                    ps = pvsb[(h, pp)]
                    rec = work.tile([1, PASS], bf16, name="recf", tag="recf")
                    with nc.allow_low_precision(reason="softmax denom bf16"):
                        nc.vector.reciprocal(rec, ps[HD:HD + 1, csl])
                    bc = fil.tile([HD, PASS], f32, name="bcf", tag="fil")
                    nc.tensor.matmul(bc, ones_row, rec, start=True, stop=True)
                    nc.vector.tensor_mul(
                        stage[b][h * HD:(h + 1) * HD, qsl], ps[0:HD, csl], bc)
                issue_a2a_half(b, p, stage[b])

            def norm_pp(pp):
                """Normalize one 1024-query chunk of both heads into the
                staging buffer.  The denominator row is round-tripped
                through DRAM so the reciprocal runs at free-size 8 (instead
                of [1, 1024], where DVE reciprocal costs ~6.4 cyc/elem) and
                the reciprocal is broadcast across partitions by the DMA
                reload; then the chunk's AllToAll is issued."""
                qsl = slice(pp * 1024, (pp + 1) * 1024)
                for h in range(2):
                    ps = pvsb[(h, pp)]
                    dr_den = dpool.tile([2 * PASS], f32, name="drden",
                                        tag="drden", bufs=2)
                    nc.sync.dma_start(dr_den, ps[HD:HD + 1, :])
                    denT = work.tile([128, 8], f32, name="denT", tag="denT")
                    nc.sync.dma_start(
                        denT, dr_den.rearrange("(p f) -> p f", p=128))
                    rcpT = work.tile([128, 8], bf16, name="rcpT", tag="rcpT")
                    with nc.allow_low_precision(reason="softmax denom bf16"):
                        nc.vector.reciprocal(rcpT, denT)
                    dr_rec = dpool.tile([2 * PASS], bf16, name="drrec",
                                        tag="drrec", bufs=2)
                    nc.sync.dma_start(
                        dr_rec.rearrange("(p f) -> p f", p=128), rcpT)
                    bcs = work.tile([HD, 2 * PASS], bf16, name="bcs",
                                    tag="bcs")
                    nc.sync.dma_start(bcs, dr_rec.partition_broadcast(HD))
                    nc.vector.tensor_mul(
                        stage[b][h * HD:(h + 1) * HD, qsl], ps[0:HD, :], bcs)
                issue_a2a(b, pp)

            def run_pending():
                while pending:
                    pending.pop(0)()

            for p in range(NPASS):
                vgen.ensure(4 * p + 3)
                q0 = p * PASS
                ks = list(range(4 * p + 4))      # visible key tiles
                pairs = []
                for i in range(0, len(ks), 2):
                    grp = ks[i:i + 2]
                    pairs.append([(k2, min(PASS, q0 + PASS - 128 * k2))
                                  for k2 in grp])
                npair = len(pairs)
                pv0 = pvp.tile([HD + 1, PASS], f32, name="pv0", tag="pv0")
                pv1 = pvp.tile([HD + 1, PASS], f32, name="pv1", tag="pv1")
                etiles = [None] * npair

                def do_pv(pi):
                    ep, widths = etiles[pi]
                    off = 0
                    for j, (k2, w) in enumerate(widths):
                        st = pi == 0 and j == 0
                        sp = pi == npair - 1 and j == len(widths) - 1
                        psl = slice(PASS - w, PASS)
                        nc.tensor.matmul(pv0[:, psl], vaug[b][:, 0, k2, :],
                                         ep[:, off:off + w],
                                         start=st, stop=sp)
                        nc.tensor.matmul(pv1[:, psl], vaug[b][:, 1, k2, :],
                                         ep[:, 1024 + off:1024 + off + w],
                                         start=st, stop=sp)
                        off += w

                for pi, widths in enumerate(pairs):
                    # scores for this pair: h0 -> columns [0, 1024), h1 ->
                    # [1024, 2048) of one 4-bank PSUM tile (row-tiled MMs of
                    # the two heads run concurrently on the PE array)
                    sp = scp.tile([128, 2048], f32, name="s", tag="s")
                    off = 0
                    for k2, w in widths:
                        kA = slice(k2 * 128, (k2 + 1) * 128)
                        qA = slice(q0 + PASS - w, q0 + PASS)
                        nc.tensor.matmul(sp[:, off:off + w],
                                         kt[b][0:HD, kA], qt[b][0:HD, qA],
                                         start=True, stop=True)
                        nc.tensor.matmul(sp[:, 1024 + off:1024 + off + w],
                                         kt[b][HD:128, kA], qt[b][HD:128, qA],
                                         start=True, stop=True)
                        off += w
                    if pi == min(2, npair - 1):
                        run_pending()
                    filler.pump(3)
                    # exp: ONE ACT instruction covers both heads' chunks
                    ep = epool.tile([128, 2048], bf16, name="e", tag="e")
                    etiles[pi] = (ep, widths)
                    if off == 1024:
                        nc.scalar.activation(ep, sp, AF.Exp, scale=0.125)
                    else:
                        nc.scalar.activation(ep[:, 0:off], sp[:, 0:off],
                                             AF.Exp, scale=0.125)
                        nc.scalar.activation(ep[:, 1024:1024 + off],
                                             sp[:, 1024:1024 + off],
                                             AF.Exp, scale=0.125)
                    # causal mask on diagonal-starting chunks
                    off = 0
                    for k2, w in widths:
                        if 128 * k2 >= q0:
                            for ho in (0, 1024):
                                nc.vector.tensor_mul(
                                    ep[:, ho + off:ho + off + 128],
                                    ep[:, ho + off:ho + off + 128], diagmask)
                        off += w
                    # PV one pair behind (exp of pair pi still in flight)
                    if pi >= 1:
                        do_pv(pi - 1)
                        filler.pump(1)
                do_pv(npair - 1)
                # free the PV accumulator banks promptly: copy to SBUF, then
                # defer normalization into a later instruction stream
                pp = p // 2
                csl = slice((p % 2) * PASS, (p % 2 + 1) * PASS)
                for h, pv in ((0, pv0), (1, pv1)):
                    if (h, pp) not in pvsb:
                        pvsb[(h, pp)] = qkp.tile(
                            [HD + 1, 2 * PASS], f32, name=f"ps{h}", tag=f"ps{h}")
                    nc.vector.tensor_copy(pvsb[(h, pp)][:, csl], pv)
                if fast_tail and p == NPASS - 2:
                    pending.append(lambda p=p: norm_half(p))
                elif fast_tail and p == NPASS - 1:
                    pass
                elif p % 2 == 1:
                    pending.append(lambda pp=pp: norm_pp(pp))
                filler.pump(2)
                if p == NPASS - 1:
                    run_pending()
                    if fast_tail:
                        a2a_filler[0].pump(16)
                        norm_fast(p)
                    else:
                        filler.pump(3)

        # ---------------- schedule ----------------
        # batch 0's V-projection tail and batch 1's projections fill PE
        # gaps during batch-0 attention; output projections (queued by
        # issue_a2a) and batch 1's V tail fill batch-1 attention.
        vgen0 = v_proj_mgen(0)
        vgen1 = v_proj_mgen(1)
        filler0 = _Filler()
        filler1 = _Filler()
        filler0.add(vgen0)
        filler0.add(qk_proj_mgen(1))
        filler1.add(vgen1)
        a2a_filler[0] = filler1
        attention(0, filler0, vgen0, fast_tail=True)
        filler0.drain()
        attention(1, filler1, vgen1, fast_tail=True)
        filler1.drain()

    nc.finalize()
    return nc


def _get_nc():
    with _lock:
        if "nc" not in _cache:
            _cache["nc"] = _build_nc()
        return _cache["nc"]


def _shard_inputs(x, input_ids, Wq, bq, Wk, bk, Wv, bv, Wo, bo):
    import ml_dtypes
    bf16 = ml_dtypes.bfloat16

    x = np.asarray(x, dtype=np.float32)
    xT = np.ascontiguousarray(x.reshape(R, D).T).astype(bf16)
    woT = np.ascontiguousarray(np.asarray(Wo, dtype=np.float32).T).astype(bf16)
    bo_f = np.asarray(bo, dtype=np.float32)
    ids = np.asarray(input_ids).astype(np.int32)
    # ids_r[p, b*NKT + t] = input_ids[b, t*128 + p]
    ids_r = np.ascontiguousarray(ids.reshape(B, NKT, 128).transpose(2, 0, 1)
                                 .reshape(128, B * NKT))
    Wq = np.asarray(Wq, dtype=np.float32)
    Wk = np.asarray(Wk, dtype=np.float32)
    Wv = np.asarray(Wv, dtype=np.float32)
    bq = np.asarray(bq, dtype=np.float32)
    bk = np.asarray(bk, dtype=np.float32)
    bv = np.asarray(bv, dtype=np.float32)

    in_maps = []
    for c in range(NCORES):
        sl = slice(c * LD, (c + 1) * LD)
        in_maps.append({
            "xT": xT,
            "wqT": np.ascontiguousarray(Wq[sl].T).astype(bf16),
            "wkT": np.ascontiguousarray(Wk[sl].T).astype(bf16),
            "wvT": np.ascontiguousarray(Wv[sl].T).astype(bf16),
            "woT": woT,
            "bq": bq[sl].copy(),
            "bk": bk[sl].copy(),
            "bv": bv[sl].copy(),
            "bo": bo_f,
            "ids": ids_r,
        })
    return in_maps


def run(trace=False, **inputs):
    """Run the kernel; returns (output, BassKernelResults)."""
    from concourse.bass_utils import run_bass_kernel_spmd

    nc = _get_nc()
    in_maps = _shard_inputs(**inputs)
    res = run_bass_kernel_spmd(nc, in_maps, core_ids=list(range(NCORES)),
                               trace=trace)
    full = np.empty((B, S, D), dtype=np.float32)
    for c in range(NCORES):
        o = np.asarray(res.results[c]["out"], dtype=np.float32)
        for b in range(B):
            full[b, c * 128:(c + 1) * 128, :] = \
                o[b * RB:b * RB + 128, :]
            # each batch's last 1024 queries travel as two per-pass half
            # A2As: core c holds queries p*512 + c*64 of passes p = 2, 3
            full[b, 1024 + c * 64:1024 + (c + 1) * 64, :] = \
                o[b * RB + 128:b * RB + 192, :]
            full[b, 1536 + c * 64:1536 + (c + 1) * 64, :] = \
                o[b * RB + 192:b * RB + 256, :]
    return full, res


def kernel(**inputs) -> np.ndarray:
    full, _ = run(trace=False, **inputs)
    return full


# revision 27
# speedup vs baseline: 1.0415x; 1.0104x over previous
"""Trainium2 Bass kernel for a causal multi-head attention layer.

Model: b=2, s=2048, d_model=1024, 16 heads, head_dim=64, pad-index 0.
Sharding over 8 NeuronCores: each core owns 2 heads (128 of the 1024
attention dims) for both batches (head/tensor parallel).  After attention,
an AllToAll redistributes the per-head outputs so each core holds all 1024
attention dims for 1/8 of the sequence positions, where it runs the output
projection locally.  Output rows per core: 256 rows of each batch.

v2 layout/schedule:
  - scores for the two local heads run concurrently on the PE array via
    row tiling (K=64 contraction at PE rows 0-63 / 64-127).
  - PV is computed "flipped" (V-with-ones-column stationary, exp(scores)
    moving), so the output lands as [dims, queries] -- no PE transposes.
    The 65th row of the PSUM accumulator is the softmax denominator.
  - attention runs in 512-query passes (exact causal staircase);
    normalization = DVE reciprocal + K=1 broadcast matmul + DVE multiply.
  - projection matmuls for the *other* batch and the output projections
    are interleaved into the attention instruction stream as PE filler so
    the tensor engine never idles (keeps the HAM clock gate at 8/8).
  - AllToAll chunks (1024 queries) are issued as soon as their passes
    finish, overlapping the remaining attention compute.
"""

import threading

import numpy as np

B, S, D = 2, 2048, 1024
H, HD = 16, 64
NCORES = 8
LD = D // NCORES          # 128 local attention dims (2 heads)
R = B * S                 # 4096 flattened rows
RC = R // NCORES          # 512 output rows per core
RB = S // NCORES          # 256 rows per batch per core
NKT = S // 128            # 16 key tiles per batch
NCH = D // 128            # 8 contraction chunks of d_model
PASS = 512                # attention query-pass width
NPASS = S // PASS         # 4 passes per batch

_cache = {}
_lock = threading.Lock()


class _MGen:
    """Ordered list of (unit, thunk) with directed catch-up: ensure(u)
    emits every thunk tagged <= u; pump_one() emits the next one."""

    def __init__(self, items):
        self.items = list(items)
        self.i = 0

    def pump_one(self):
        if self.i < len(self.items):
            self.items[self.i][1]()
            self.i += 1
            return True
        return False

    def ensure(self, unit):
        while self.i < len(self.items) and self.items[self.i][0] <= unit:
            self.items[self.i][1]()
            self.i += 1


class _Filler:
    """FIFO of thunk sources, pumped between attention ops to keep the
    tensor engine busy during softmax-bound stretches.  A source added
    with delay=k stays locked for the next k pump ticks, so thunks whose
    data dependencies are still in flight (e.g. an output projection
    whose AllToAll hasn't landed) don't block the in-order PE queue."""

    def __init__(self):
        self.srcs = []
        self.tick = 0

    def add(self, src, delay=0):
        if not isinstance(src, _MGen):
            src = _MGen([(0, t) for t in src])
        self.srcs.append((self.tick + delay, src))

    def pump(self, n):
        while n > 0:
            self.tick += 1
            n -= 1
            for j, (ready_at, src) in enumerate(self.srcs):
                if ready_at > self.tick:
                    continue
                if src.pump_one():
                    break
                else:
                    self.srcs.pop(j)
                    break
            else:
                return

    def drain(self):
        while self.srcs:
            changed = False
            for ready_at, src in list(self.srcs):
                if src.pump_one():
                    changed = True
                else:
                    self.srcs = [(r, s) for r, s in self.srcs if s is not src]
            if not changed and not self.srcs:
                break


def _build_nc():
    import concourse.mybir as mybir
    import concourse.tile as tile
    from concourse import bacc
    from contextlib import ExitStack

    f32 = mybir.dt.float32
    bf16 = mybir.dt.bfloat16
    i32 = mybir.dt.int32
    AF = mybir.ActivationFunctionType
    ALU = mybir.AluOpType

    nc = bacc.Bacc(None, target_bir_lowering=False, num_devices=NCORES)

    xT = nc.declare_dram_parameter("xT", [D, R], bf16, isOutput=False)
    wqT = nc.declare_dram_parameter("wqT", [D, LD], bf16, isOutput=False)
    wkT = nc.declare_dram_parameter("wkT", [D, LD], bf16, isOutput=False)
    wvT = nc.declare_dram_parameter("wvT", [D, LD], bf16, isOutput=False)
    woT = nc.declare_dram_parameter("woT", [D, D], bf16, isOutput=False)
    bq = nc.declare_dram_parameter("bq", [LD], f32, isOutput=False)
    bk = nc.declare_dram_parameter("bk", [LD], f32, isOutput=False)
    bv = nc.declare_dram_parameter("bv", [LD], f32, isOutput=False)
    bo = nc.declare_dram_parameter("bo", [D], f32, isOutput=False)
    ids = nc.declare_dram_parameter("ids", [128, B * NKT], i32, isOutput=False)
    out = nc.declare_dram_parameter("out", [RC, D], f32, isOutput=True)

    with ExitStack() as ctx:
        tc = ctx.enter_context(tile.TileContext(nc))
        const = ctx.enter_context(tc.tile_pool(name="const", bufs=1))
        qkp = ctx.enter_context(tc.tile_pool(name="qkp", bufs=2))
        work = ctx.enter_context(tc.tile_pool(name="work", bufs=4))
        epool = ctx.enter_context(tc.tile_pool(name="epool", bufs=3))
        stg = ctx.enter_context(tc.tile_pool(name="stg", bufs=2))
        dpool = ctx.enter_context(tc.tile_pool(name="dram", bufs=2, space="DRAM"))

        # ---- constants (small weights first so compute can start early) ----
        wqT_sb = const.tile([128, NCH, LD], bf16)
        nc.sync.dma_start(wqT_sb, wqT.ap().rearrange("(c p) d -> p c d", p=128))
        wkT_sb = const.tile([128, NCH, LD], bf16)
        nc.sync.dma_start(wkT_sb, wkT.ap().rearrange("(c p) d -> p c d", p=128))
        wvT_sb = const.tile([128, NCH, LD], bf16)
        nc.sync.dma_start(wvT_sb, wvT.ap().rearrange("(c p) d -> p c d", p=128))
        bq_col = const.tile([128, 1], f32)
        nc.sync.dma_start(bq_col, bq.ap().rearrange("(p o) -> p o", o=1))
        bk_col = const.tile([128, 1], f32)
        nc.sync.dma_start(bk_col, bk.ap().rearrange("(p o) -> p o", o=1))
        bv_bc = const.tile([128, LD], f32)
        nc.sync.dma_start(bv_bc, bv.ap().partition_broadcast(128))
        ids_sb = const.tile([128, B * NKT], i32)
        nc.sync.dma_start(ids_sb, ids.ap())

        # x^T per (batch, contraction chunk): [128, S] tiles.  Batch 0's
        # chunks first so its QK projection starts after the first lands.
        xTr = xT.ap().rearrange("(c p) (b r) -> b c p r", p=128, b=B)
        xb = [[None] * NCH for _ in range(B)]
        for b in range(B):
            for c in range(NCH):
                t = const.tile([128, S], bf16, name=f"x{b}c{c}", tag=f"x{b}c{c}")
                # split the 8.4MB of x across two DMA queues (the scalar
                # engine is idle until the first exp) so the projection
                # pipeline isn't paced by a single serialized queue
                eng = nc.sync if c % 2 == 0 else nc.scalar
                eng.dma_start(t, xTr[b, c])
                xb[b][c] = t
        woT_sb = const.tile([128, NCH, D], bf16)
        nc.scalar.dma_start(woT_sb,
                            woT.ap().rearrange("(c p) n -> p c n", p=128))
        bo_bc = const.tile([128, D], f32)
        nc.scalar.dma_start(bo_bc, bo.ap().partition_broadcast(128))

        padf = const.tile([128, B * NKT], f32)
        nc.vector.tensor_copy(padf, ids_sb)
        nc.vector.tensor_scalar_min(padf, padf, 1.0)

        ones_row = const.tile([1, HD], bf16)
        nc.gpsimd.memset(ones_row, 1.0)
        # diagmask[x, y] = 1 if y >= x else 0  (keys on partitions)
        diagmask = const.tile([128, 128], bf16)
        nc.gpsimd.memset(diagmask, 1.0)
        nc.gpsimd.affine_select(
            out=diagmask, in_=diagmask, compare_op=ALU.is_ge, fill=0.0,
            base=0, pattern=[[1, 128]], channel_multiplier=-1,
        )

        qt = [None] * B
        kt = [None] * B
        vaug = [None] * B
        stage = [None] * B

        # ---- batch 0 QK projection, contraction-outer (DMA-pipelined) ----
        # Uses 8 PSUM banks transiently; the pool closes before the
        # attention-phase PSUM pools open.
        qt[0] = qkp.tile([128, S], bf16, name="qt0", tag="qt")
        kt[0] = qkp.tile([128, S], bf16, name="kt0", tag="kt")
        with tc.tile_pool(name="qk8", bufs=1, space="PSUM") as qk8:
            pq = qk8.tile([128, S], f32, name="pq0", tag="pq")
            pk = qk8.tile([128, S], f32, name="pk0", tag="pk")
            for c in range(NCH):
                st, sp = c == 0, c == NCH - 1
                for q4 in range(S // 512):
                    sl = slice(q4 * 512, (q4 + 1) * 512)
                    nc.tensor.matmul(pq[:, sl], wqT_sb[:, c, :],
                                     xb[0][c][:, sl], start=st, stop=sp)
                    nc.tensor.matmul(pk[:, sl], wkT_sb[:, c, :],
                                     xb[0][c][:, sl], start=st, stop=sp)
            for q4 in range(S // 512):
                sl = slice(q4 * 512, (q4 + 1) * 512)
                nc.vector.tensor_scalar_add(qt[0][:, sl], pq[:, sl], bq_col)
                nc.vector.tensor_scalar_add(kt[0][:, sl], pk[:, sl], bk_col)

        # ---- attention-phase PSUM pools (exactly 8 banks) ----
        sc0 = ctx.enter_context(tc.tile_pool(name="sc0", bufs=1, space="PSUM"))
        sc1 = ctx.enter_context(tc.tile_pool(name="sc1", bufs=1, space="PSUM"))
        pvp = ctx.enter_context(tc.tile_pool(name="pvp", bufs=1, space="PSUM"))
        fil = ctx.enter_context(tc.tile_pool(name="fil", bufs=2, space="PSUM"))

        def v_proj_mgen(b):
            """V projection in [keys, dims] layout + bias/pad/ones -> vaug.
            Two thunks per key tile m (4 contraction chunks each); unit = m
            so attention passes can ensure() the tiles they need."""
            vaug[b] = qkp.tile([128, 2, NKT, HD + 1], bf16,
                               name=f"vaug{b}", tag="vaug")
            items = []
            for m in range(NKT):
                hold = [None]

                def goA(m=m, b=b, hold=hold):
                    rsl = slice(m * 128, (m + 1) * 128)
                    hold[0] = fil.tile([128, 512], f32, name="pv", tag="fil")
                    for c in range(4):
                        nc.tensor.matmul(hold[0][:, 0:LD], xb[b][c][:, rsl],
                                         wvT_sb[:, c, :],
                                         start=(c == 0), stop=False)

                def goB(m=m, b=b, hold=hold):
                    rsl = slice(m * 128, (m + 1) * 128)
                    pvt = hold[0]
                    for c in range(4, NCH):
                        nc.tensor.matmul(pvt[:, 0:LD], xb[b][c][:, rsl],
                                         wvT_sb[:, c, :],
                                         start=False, stop=(c == NCH - 1))
                    tv = work.tile([128, LD], f32, name="tv", tag="tv")
                    nc.vector.tensor_add(tv, pvt[:, 0:LD], bv_bc)
                    pcol = padf[:, b * NKT + m:b * NKT + m + 1]
                    for h in range(2):
                        nc.vector.tensor_scalar_mul(
                            vaug[b][:, h, m, 0:HD], tv[:, h * HD:(h + 1) * HD],
                            pcol)
                        nc.vector.tensor_copy(vaug[b][:, h, m, HD:HD + 1], pcol)
                items.append((m, goA))
                items.append((m, goB))
            return _MGen(items)

        def qk_proj_mgen(b):
            """QK projection as filler thunks (two per 512-row quarter per
            q/k; x for batch b must be resident when these run)."""
            qt[b] = qkp.tile([128, S], bf16, name=f"qt{b}", tag="qt")
            kt[b] = qkp.tile([128, S], bf16, name=f"kt{b}", tag="kt")
            items = []
            for q4 in range(S // 512):
                for wsb, bcol, dstl in ((wqT_sb, bq_col, qt),
                                        (wkT_sb, bk_col, kt)):
                    hold = [None]

                    def goA(q4=q4, b=b, wsb=wsb, hold=hold):
                        sl = slice(q4 * 512, (q4 + 1) * 512)
                        hold[0] = fil.tile([128, 512], f32, name="pq",
                                           tag="fil")
                        for c in range(4):
                            nc.tensor.matmul(hold[0], wsb[:, c, :],
                                             xb[b][c][:, sl],
                                             start=(c == 0), stop=False)

                    def goB(q4=q4, b=b, wsb=wsb, bcol=bcol, dstl=dstl,
                            hold=hold):
                        sl = slice(q4 * 512, (q4 + 1) * 512)
                        for c in range(4, NCH):
                            nc.tensor.matmul(hold[0], wsb[:, c, :],
                                             xb[b][c][:, sl],
                                             start=False, stop=(c == NCH - 1))
                        nc.vector.tensor_scalar_add(dstl[b][:, sl], hold[0],
                                                    bcol)
                    items.append((q4, goA))
                    items.append((q4, goB))
            return _MGen(items)

        def outproj_thunks(b, pp, a2a_out):
            """Output projection for one gathered 128-row chunk (the final
            output rows ride the sync DMA queue)."""
            a2a_sb = stg.tile([128, NCORES, 128], bf16, name=f"a2as{b}{pp}",
                              tag="a2as", bufs=4)

            def load(a2a_sb=a2a_sb, a2a_out=a2a_out):
                nc.sync.dma_start(
                    a2a_sb, a2a_out.rearrange("(j p) r -> p j r", p=128))
            yield load
            r0 = b * RB + pp * 128
            for n in range(D // 512):
                hold = [None]

                def goA(n=n, a2a_sb=a2a_sb, hold=hold):
                    hold[0] = fil.tile([128, 512], f32, name="po", tag="fil")
                    for c in range(4):
                        nc.tensor.matmul(
                            hold[0], a2a_sb[:, c, :],
                            woT_sb[:, c, n * 512:(n + 1) * 512],
                            start=(c == 0), stop=False)
                yield goA

                def goB(n=n, a2a_sb=a2a_sb, r0=r0, hold=hold):
                    for c in range(4, NCH):
                        nc.tensor.matmul(
                            hold[0], a2a_sb[:, c, :],
                            woT_sb[:, c, n * 512:(n + 1) * 512],
                            start=False, stop=(c == NCH - 1))
                    ot = work.tile([128, 512], f32, name="ot", tag="ot")
                    nc.vector.tensor_add(ot, hold[0],
                                         bo_bc[:, n * 512:(n + 1) * 512])
                    nc.sync.dma_start(
                        out.ap()[r0:r0 + 128, n * 512:(n + 1) * 512], ot)
                yield goB

        a2a_filler = [None]

        def outproj_half_thunks(b, p, a2a_out):
            """Output projection for one gathered 64-row (half) chunk."""
            a2a_sb = stg.tile([128, NCORES, 64], bf16, name=f"a2ah{b}{p}",
                              tag="a2ah", bufs=2)

            def load(a2a_sb=a2a_sb, a2a_out=a2a_out):
                nc.sync.dma_start(
                    a2a_sb, a2a_out.rearrange("(j p) r -> p j r", p=128))
            yield load
            r0 = b * RB + (p // 2) * 128 + (p % 2) * 64
            for n in range(D // 512):
                def go(n=n, a2a_sb=a2a_sb, r0=r0):
                    pout = fil.tile([128, 512], f32, name="po", tag="fil")
                    for c in range(NCH):
                        nc.tensor.matmul(
                            pout[0:HD, :], a2a_sb[:, c, :],
                            woT_sb[:, c, n * 512:(n + 1) * 512],
                            start=(c == 0), stop=(c == NCH - 1))
                    ot = work.tile([HD, 512], f32, name="oth", tag="oth")
                    nc.vector.tensor_add(ot, pout[0:HD, :],
                                         bo_bc[0:HD, n * 512:(n + 1) * 512])
                    nc.sync.dma_start(
                        out.ap()[r0:r0 + HD, n * 512:(n + 1) * 512], ot)
                yield go

        def issue_a2a_half(b, p, stage_b):
            """AllToAll a single 512-query pass (64 queries per core) so the
            final chunk's latency chain is as short as possible."""
            a2a_in = dpool.tile([NCORES * 128, 64], bf16,
                                name=f"a2aih{b}{p}", tag="a2aih", bufs=2)
            nc.sync.dma_start(
                a2a_in.rearrange("(j p) r -> p j r", p=128),
                stage_b[:, p * PASS:(p + 1) * PASS]
                .rearrange("p (j r) -> p j r", j=NCORES))
            a2a_out = dpool.tile([NCORES * 128, 64], bf16,
                                 name=f"a2aoh{b}{p}", tag="a2aoh", bufs=2)
            nc.gpsimd.collective_compute(
                "AllToAll", ALU.bypass,
                replica_groups=[list(range(NCORES))],
                ins=[a2a_in.opt()], outs=[a2a_out.opt()])
            a2a_filler[0].add(outproj_half_thunks(b, p, a2a_out), delay=14)

        def issue_a2a(b, pp):
            """AllToAll one 1024-query chunk of batch b's stage buffer; its
            output projection becomes filler work.  The (1, 0) chunk's
            projection is reserved (large delay) so it fills the PE while
            the final chunk's AllToAll drains peer skew."""
            a2a_in = dpool.tile([NCORES * 128, 128], bf16,
                                name=f"a2ai{b}{pp}", tag="a2ai", bufs=4)
            nc.sync.dma_start(
                a2a_in.rearrange("(j p) r -> p j r", p=128),
                stage[b][:, pp * 1024:(pp + 1) * 1024]
                .rearrange("p (j r) -> p j r", j=NCORES))
            a2a_out = dpool.tile([NCORES * 128, 128], bf16,
                                 name=f"a2ao{b}{pp}", tag="a2ao", bufs=4)
            nc.gpsimd.collective_compute(
                "AllToAll", ALU.bypass,
                replica_groups=[list(range(NCORES))],
                ins=[a2a_in.opt()], outs=[a2a_out.opt()])
            if b == 1 and pp == 0:
                # reserve this chunk's matmuls for the kernel tail (they
                # fill the PE while the final chunk's AllToAll completes),
                # but let its gather-load DMA go out early so it isn't
                # queued behind the final normalization chain
                thunks = list(outproj_thunks(b, pp, a2a_out))
                a2a_filler[0].add(thunks[:1], delay=14)
                a2a_filler[0].add(thunks[1:], delay=44)
            else:
                a2a_filler[0].add(outproj_thunks(b, pp, a2a_out), delay=14)

        def attention(b, filler, vgen, fast_tail=False):
            """Attention for batch b, both heads, 512-query passes.  PV is
            pipelined one score-pair behind exp.  Normalization for a
            1024-query chunk is deferred into the next pass so the PE queue
            never waits on the reciprocal chain."""
            stage[b] = stg.tile([128, S], bf16, name=f"stage{b}", tag="stage")
            pvsb = {}            # (h, pp) -> [65, 1024] f32 SBUF copy of PV
            pending = []         # deferred normalization closures

            def norm_half(p):
                """Per-pass DMA-chain normalization + half A2A (used for the
                next-to-last pass of the fast tail)."""
                pp = p // 2
                csl = slice((p % 2) * PASS, (p % 2 + 1) * PASS)
                qsl = slice(p * PASS, (p + 1) * PASS)
                for h in range(2):
                    ps = pvsb[(h, pp)]
                    dr_den = dpool.tile([PASS], f32, name="drdh",
                                        tag="drdh", bufs=2)
                    nc.sync.dma_start(dr_den, ps[HD:HD + 1, csl])
                    denT = work.tile([128, 4], f32, name="denTh", tag="denTh")
                    nc.sync.dma_start(
                        denT, dr_den.rearrange("(p f) -> p f", p=128))
                    rcpT = work.tile([128, 4], bf16, name="rcpTh", tag="rcpTh")
                    with nc.allow_low_precision(reason="softmax denom bf16"):
                        nc.vector.reciprocal(rcpT, denT)
                    dr_rec = dpool.tile([PASS], bf16, name="drrh",
                                        tag="drrh", bufs=2)
                    nc.sync.dma_start(
                        dr_rec.rearrange("(p f) -> p f", p=128), rcpT)
                    bcs = work.tile([HD, PASS], bf16, name="bcsh", tag="bcsh")
                    nc.sync.dma_start(bcs, dr_rec.partition_broadcast(HD))
                    nc.vector.tensor_mul(
                        stage[b][h * HD:(h + 1) * HD, qsl], ps[0:HD, csl], bcs)
                issue_a2a_half(b, p, stage[b])

            def norm_fast(p):
                """Latency-optimal normalization for the very last pass:
                direct DVE reciprocal + ones-matmul broadcast (no DMA round
                trips), then the half A2A."""
                pp = p // 2
                csl = slice((p % 2) * PASS, (p % 2 + 1) * PASS)
                qsl = slice(p * PASS, (p + 1) * PASS)
                for h in range(2):
                    ps = pvsb[(h, pp)]
                    rec = work.tile([1, PASS], bf16, name="recf", tag="recf")
                    with nc.allow_low_precision(reason="softmax denom bf16"):
                        nc.vector.reciprocal(rec, ps[HD:HD + 1, csl])
                    bc = fil.tile([HD, PASS], f32, name="bcf", tag="fil")
                    nc.tensor.matmul(bc, ones_row, rec, start=True, stop=True)
                    nc.vector.tensor_mul(
                        stage[b][h * HD:(h + 1) * HD, qsl], ps[0:HD, csl], bc)
                issue_a2a_half(b, p, stage[b])

            def norm_pp(pp):
                """Normalize one 1024-query chunk of both heads into the
                staging buffer.  The denominator row is round-tripped
                through DRAM so the reciprocal runs at free-size 8 (instead
                of [1, 1024], where DVE reciprocal costs ~6.4 cyc/elem) and
                the reciprocal is broadcast across partitions by the DMA
                reload; then the chunk's AllToAll is issued."""
                qsl = slice(pp * 1024, (pp + 1) * 1024)
                for h in range(2):
                    ps = pvsb[(h, pp)]
                    dr_den = dpool.tile([2 * PASS], f32, name="drden",
                                        tag="drden", bufs=2)
                    nc.sync.dma_start(dr_den, ps[HD:HD + 1, :])
                    denT = work.tile([128, 8], f32, name="denT", tag="denT")
                    nc.sync.dma_start(
                        denT, dr_den.rearrange("(p f) -> p f", p=128))
                    rcpT = work.tile([128, 8], bf16, name="rcpT", tag="rcpT")
                    with nc.allow_low_precision(reason="softmax denom bf16"):
                        nc.vector.reciprocal(rcpT, denT)
                    dr_rec = dpool.tile([2 * PASS], bf16, name="drrec",
                                        tag="drrec", bufs=2)
                    nc.sync.dma_start(
                        dr_rec.rearrange("(p f) -> p f", p=128), rcpT)
                    bcs = work.tile([HD, 2 * PASS], bf16, name="bcs",
                                    tag="bcs")
                    nc.sync.dma_start(bcs, dr_rec.partition_broadcast(HD))
                    nc.vector.tensor_mul(
                        stage[b][h * HD:(h + 1) * HD, qsl], ps[0:HD, :], bcs)
                issue_a2a(b, pp)

            def run_pending():
                while pending:
                    pending.pop(0)()

            for p in range(NPASS):
                vgen.ensure(4 * p + 3)
                q0 = p * PASS
                ks = list(range(4 * p + 4))      # visible key tiles
                pairs = []
                for i in range(0, len(ks), 2):
                    grp = ks[i:i + 2]
                    pairs.append([(k2, min(PASS, q0 + PASS - 128 * k2))
                                  for k2 in grp])
                npair = len(pairs)
                pv0 = pvp.tile([HD + 1, PASS], f32, name="pv0", tag="pv0")
                pv1 = pvp.tile([HD + 1, PASS], f32, name="pv1", tag="pv1")
                etiles = [None] * npair

                def do_pv(pi):
                    ep, widths = etiles[pi]
                    off = 0
                    for j, (k2, w) in enumerate(widths):
                        st = pi == 0 and j == 0
                        sp = pi == npair - 1 and j == len(widths) - 1
                        psl = slice(PASS - w, PASS)
                        nc.tensor.matmul(pv0[:, psl], vaug[b][:, 0, k2, :],
                                         ep[:, off:off + w],
                                         start=st, stop=sp)
                        nc.tensor.matmul(pv1[:, psl], vaug[b][:, 1, k2, :],
                                         ep[:, 1024 + off:1024 + off + w],
                                         start=st, stop=sp)
                        off += w

                for pi, widths in enumerate(pairs):
                    # scores for this pair, both heads (row-tiled MMs of the
                    # two heads run concurrently on the PE array; separate
                    # per-head PSUM slots keep exp/scores pipelined)
                    sp0 = sc0.tile([128, 1024], f32, name="s0", tag="s0")
                    sp1 = sc1.tile([128, 1024], f32, name="s1", tag="s1")
                    off = 0
                    for k2, w in widths:
                        kA = slice(k2 * 128, (k2 + 1) * 128)
                        qA = slice(q0 + PASS - w, q0 + PASS)
                        nc.tensor.matmul(sp0[:, off:off + w],
                                         kt[b][0:HD, kA], qt[b][0:HD, qA],
                                         start=True, stop=True)
                        nc.tensor.matmul(sp1[:, off:off + w],
                                         kt[b][HD:128, kA], qt[b][HD:128, qA],
                                         start=True, stop=True)
                        off += w
                    if pi == min(2, npair - 1):
                        run_pending()
                    filler.pump(3)
                    # exp (one ACT instruction per head per pair)
                    ep = epool.tile([128, 2048], bf16, name="e", tag="e")
                    etiles[pi] = (ep, widths)
                    nc.scalar.activation(ep[:, 0:off], sp0[:, 0:off],
                                         AF.Exp, scale=0.125)
                    nc.scalar.activation(ep[:, 1024:1024 + off],
                                         sp1[:, 0:off], AF.Exp, scale=0.125)
                    # causal mask on diagonal-starting chunks
                    off = 0
                    for k2, w in widths:
                        if 128 * k2 >= q0:
                            for ho in (0, 1024):
                                nc.vector.tensor_mul(
                                    ep[:, ho + off:ho + off + 128],
                                    ep[:, ho + off:ho + off + 128], diagmask)
                        off += w
                    # PV one pair behind (exp of pair pi still in flight)
                    if pi >= 1:
                        do_pv(pi - 1)
                        filler.pump(1)
                do_pv(npair - 1)
                # free the PV accumulator banks promptly: copy to SBUF, then
                # defer normalization into a later instruction stream
                pp = p // 2
                csl = slice((p % 2) * PASS, (p % 2 + 1) * PASS)
                for h, pv in ((0, pv0), (1, pv1)):
                    if (h, pp) not in pvsb:
                        pvsb[(h, pp)] = qkp.tile(
                            [HD + 1, 2 * PASS], f32, name=f"ps{h}", tag=f"ps{h}")
                    nc.vector.tensor_copy(pvsb[(h, pp)][:, csl], pv)
                if fast_tail and p == NPASS - 2:
                    pending.append(lambda p=p: norm_half(p))
                elif fast_tail and p == NPASS - 1:
                    pass
                elif p % 2 == 1:
                    pending.append(lambda pp=pp: norm_pp(pp))
                filler.pump(2)
                if p == NPASS - 1:
                    run_pending()
                    if fast_tail:
                        a2a_filler[0].pump(16)
                        norm_fast(p)
                    else:
                        filler.pump(3)

        # ---------------- schedule ----------------
        # batch 0's V-projection tail and batch 1's projections fill PE
        # gaps during batch-0 attention; output projections (queued by
        # issue_a2a) and batch 1's V tail fill batch-1 attention.
        vgen0 = v_proj_mgen(0)
        vgen1 = v_proj_mgen(1)
        filler0 = _Filler()
        filler1 = _Filler()
        filler0.add(vgen0)
        filler0.add(qk_proj_mgen(1))
        filler1.add(vgen1)
        a2a_filler[0] = filler1
        attention(0, filler0, vgen0, fast_tail=True)
        filler0.drain()
        attention(1, filler1, vgen1, fast_tail=True)
        filler1.drain()

    nc.finalize()
    return nc


def _get_nc():
    with _lock:
        if "nc" not in _cache:
            _cache["nc"] = _build_nc()
        return _cache["nc"]


def _shard_inputs(x, input_ids, Wq, bq, Wk, bk, Wv, bv, Wo, bo):
    import ml_dtypes
    bf16 = ml_dtypes.bfloat16

    x = np.asarray(x, dtype=np.float32)
    xT = np.ascontiguousarray(x.reshape(R, D).T).astype(bf16)
    woT = np.ascontiguousarray(np.asarray(Wo, dtype=np.float32).T).astype(bf16)
    bo_f = np.asarray(bo, dtype=np.float32)
    ids = np.asarray(input_ids).astype(np.int32)
    # ids_r[p, b*NKT + t] = input_ids[b, t*128 + p]
    ids_r = np.ascontiguousarray(ids.reshape(B, NKT, 128).transpose(2, 0, 1)
                                 .reshape(128, B * NKT))
    Wq = np.asarray(Wq, dtype=np.float32)
    Wk = np.asarray(Wk, dtype=np.float32)
    Wv = np.asarray(Wv, dtype=np.float32)
    bq = np.asarray(bq, dtype=np.float32)
    bk = np.asarray(bk, dtype=np.float32)
    bv = np.asarray(bv, dtype=np.float32)

    in_maps = []
    for c in range(NCORES):
        sl = slice(c * LD, (c + 1) * LD)
        in_maps.append({
            "xT": xT,
            "wqT": np.ascontiguousarray(Wq[sl].T).astype(bf16),
            "wkT": np.ascontiguousarray(Wk[sl].T).astype(bf16),
            "wvT": np.ascontiguousarray(Wv[sl].T).astype(bf16),
            "woT": woT,
            "bq": bq[sl].copy(),
            "bk": bk[sl].copy(),
            "bv": bv[sl].copy(),
            "bo": bo_f,
            "ids": ids_r,
        })
    return in_maps


def run(trace=False, **inputs):
    """Run the kernel; returns (output, BassKernelResults)."""
    from concourse.bass_utils import run_bass_kernel_spmd

    nc = _get_nc()
    in_maps = _shard_inputs(**inputs)
    res = run_bass_kernel_spmd(nc, in_maps, core_ids=list(range(NCORES)),
                               trace=trace)
    full = np.empty((B, S, D), dtype=np.float32)
    for c in range(NCORES):
        o = np.asarray(res.results[c]["out"], dtype=np.float32)
        for b in range(B):
            full[b, c * 128:(c + 1) * 128, :] = \
                o[b * RB:b * RB + 128, :]
            # each batch's last 1024 queries travel as two per-pass half
            # A2As: core c holds queries p*512 + c*64 of passes p = 2, 3
            full[b, 1024 + c * 64:1024 + (c + 1) * 64, :] = \
                o[b * RB + 128:b * RB + 192, :]
            full[b, 1536 + c * 64:1536 + (c + 1) * 64, :] = \
                o[b * RB + 192:b * RB + 256, :]
    return full, res


def kernel(**inputs) -> np.ndarray:
    full, _ = run(trace=False, **inputs)
    return full


# revision 28
# speedup vs baseline: 1.0744x; 1.0316x over previous
"""Trainium2 Bass kernel for a causal multi-head attention layer.

Model: b=2, s=2048, d_model=1024, 16 heads, head_dim=64, pad-index 0.
Sharding over 8 NeuronCores: each core owns 2 heads (128 of the 1024
attention dims) for both batches (head/tensor parallel).  After attention,
an AllToAll redistributes the per-head outputs so each core holds all 1024
attention dims for 1/8 of the sequence positions, where it runs the output
projection locally.  Output rows per core: 256 rows of each batch.

v2 layout/schedule:
  - scores for the two local heads run concurrently on the PE array via
    row tiling (K=64 contraction at PE rows 0-63 / 64-127).
  - PV is computed "flipped" (V-with-ones-column stationary, exp(scores)
    moving), so the output lands as [dims, queries] -- no PE transposes.
    The 65th row of the PSUM accumulator is the softmax denominator.
  - attention runs in 512-query passes (exact causal staircase);
    normalization = DVE reciprocal + K=1 broadcast matmul + DVE multiply.
  - projection matmuls for the *other* batch and the output projections
    are interleaved into the attention instruction stream as PE filler so
    the tensor engine never idles (keeps the HAM clock gate at 8/8).
  - AllToAll chunks (1024 queries) are issued as soon as their passes
    finish, overlapping the remaining attention compute.
"""

import threading

import numpy as np

B, S, D = 2, 2048, 1024
H, HD = 16, 64
NCORES = 8
LD = D // NCORES          # 128 local attention dims (2 heads)
R = B * S                 # 4096 flattened rows
RC = R // NCORES          # 512 output rows per core
RB = S // NCORES          # 256 rows per batch per core
NKT = S // 128            # 16 key tiles per batch
NCH = D // 128            # 8 contraction chunks of d_model
PASS = 512                # attention query-pass width
NPASS = S // PASS         # 4 passes per batch

_cache = {}
_lock = threading.Lock()


class _MGen:
    """Ordered list of (unit, thunk) with directed catch-up: ensure(u)
    emits every thunk tagged <= u; pump_one() emits the next one."""

    def __init__(self, items):
        self.items = list(items)
        self.i = 0

    def pump_one(self):
        if self.i < len(self.items):
            self.items[self.i][1]()
            self.i += 1
            return True
        return False

    def ensure(self, unit):
        while self.i < len(self.items) and self.items[self.i][0] <= unit:
            self.items[self.i][1]()
            self.i += 1


class _Filler:
    """FIFO of thunk sources, pumped between attention ops to keep the
    tensor engine busy during softmax-bound stretches.  A source added
    with delay=k stays locked for the next k pump ticks, so thunks whose
    data dependencies are still in flight (e.g. an output projection
    whose AllToAll hasn't landed) don't block the in-order PE queue."""

    def __init__(self):
        self.srcs = []
        self.tick = 0

    def add(self, src, delay=0):
        if not isinstance(src, _MGen):
            src = _MGen([(0, t) for t in src])
        self.srcs.append((self.tick + delay, src))

    def pump(self, n):
        while n > 0:
            self.tick += 1
            n -= 1
            for j, (ready_at, src) in enumerate(self.srcs):
                if ready_at > self.tick:
                    continue
                if src.pump_one():
                    break
                else:
                    self.srcs.pop(j)
                    break
            else:
                return

    def drain(self):
        while self.srcs:
            changed = False
            for ready_at, src in list(self.srcs):
                if src.pump_one():
                    changed = True
                else:
                    self.srcs = [(r, s) for r, s in self.srcs if s is not src]
            if not changed and not self.srcs:
                break


def _build_nc():
    import concourse.mybir as mybir
    import concourse.tile as tile
    from concourse import bacc
    from contextlib import ExitStack

    f32 = mybir.dt.float32
    bf16 = mybir.dt.bfloat16
    i32 = mybir.dt.int32
    AF = mybir.ActivationFunctionType
    ALU = mybir.AluOpType

    nc = bacc.Bacc(None, target_bir_lowering=False, num_devices=NCORES)

    xT = nc.declare_dram_parameter("xT", [D, R], bf16, isOutput=False)
    wqT = nc.declare_dram_parameter("wqT", [D, LD], bf16, isOutput=False)
    wkT = nc.declare_dram_parameter("wkT", [D, LD], bf16, isOutput=False)
    wvT = nc.declare_dram_parameter("wvT", [D, LD], bf16, isOutput=False)
    woT = nc.declare_dram_parameter("woT", [D, D], bf16, isOutput=False)
    bq = nc.declare_dram_parameter("bq", [LD], f32, isOutput=False)
    bk = nc.declare_dram_parameter("bk", [LD], f32, isOutput=False)
    bv = nc.declare_dram_parameter("bv", [LD], f32, isOutput=False)
    bo = nc.declare_dram_parameter("bo", [D], f32, isOutput=False)
    ids = nc.declare_dram_parameter("ids", [128, B * NKT], i32, isOutput=False)
    out = nc.declare_dram_parameter("out", [RC, D], f32, isOutput=True)

    with ExitStack() as ctx:
        tc = ctx.enter_context(tile.TileContext(nc))
        const = ctx.enter_context(tc.tile_pool(name="const", bufs=1))
        qkp = ctx.enter_context(tc.tile_pool(name="qkp", bufs=2))
        work = ctx.enter_context(tc.tile_pool(name="work", bufs=4))
        epool = ctx.enter_context(tc.tile_pool(name="epool", bufs=3))
        stg = ctx.enter_context(tc.tile_pool(name="stg", bufs=2))
        dpool = ctx.enter_context(tc.tile_pool(name="dram", bufs=2, space="DRAM"))

        # ---- constants (small weights first so compute can start early) ----
        wqT_sb = const.tile([128, NCH, LD], bf16)
        nc.sync.dma_start(wqT_sb, wqT.ap().rearrange("(c p) d -> p c d", p=128))
        wkT_sb = const.tile([128, NCH, LD], bf16)
        nc.sync.dma_start(wkT_sb, wkT.ap().rearrange("(c p) d -> p c d", p=128))
        wvT_sb = const.tile([128, NCH, LD], bf16)
        nc.sync.dma_start(wvT_sb, wvT.ap().rearrange("(c p) d -> p c d", p=128))
        bq_col = const.tile([128, 1], f32)
        nc.sync.dma_start(bq_col, bq.ap().rearrange("(p o) -> p o", o=1))
        bk_col = const.tile([128, 1], f32)
        nc.sync.dma_start(bk_col, bk.ap().rearrange("(p o) -> p o", o=1))
        bv_bc = const.tile([128, LD], f32)
        nc.sync.dma_start(bv_bc, bv.ap().partition_broadcast(128))
        ids_sb = const.tile([128, B * NKT], i32)
        nc.sync.dma_start(ids_sb, ids.ap())

        # x^T per (batch, contraction chunk): [128, S] tiles.  Batch 0's
        # chunks first so its QK projection starts after the first lands.
        xTr = xT.ap().rearrange("(c p) (b r) -> b c p r", p=128, b=B)
        xb = [[None] * NCH for _ in range(B)]
        for b in range(B):
            for c in range(NCH):
                t = const.tile([128, S], bf16, name=f"x{b}c{c}", tag=f"x{b}c{c}")
                # split the 8.4MB of x across two DMA queues (the scalar
                # engine is idle until the first exp) so the projection
                # pipeline isn't paced by a single serialized queue
                eng = nc.sync if c % 2 == 0 else nc.scalar
                eng.dma_start(t, xTr[b, c])
                xb[b][c] = t
        woT_sb = const.tile([128, NCH, D], bf16)
        nc.scalar.dma_start(woT_sb,
                            woT.ap().rearrange("(c p) n -> p c n", p=128))
        bo_bc = const.tile([128, D], f32)
        nc.scalar.dma_start(bo_bc, bo.ap().partition_broadcast(128))

        padf = const.tile([128, B * NKT], f32)
        nc.vector.tensor_copy(padf, ids_sb)
        nc.vector.tensor_scalar_min(padf, padf, 1.0)

        ones_row = const.tile([1, HD], bf16)
        nc.gpsimd.memset(ones_row, 1.0)
        # diagmask[x, y] = 1 if y >= x else 0  (keys on partitions)
        diagmask = const.tile([128, 128], bf16)
        nc.gpsimd.memset(diagmask, 1.0)
        nc.gpsimd.affine_select(
            out=diagmask, in_=diagmask, compare_op=ALU.is_ge, fill=0.0,
            base=0, pattern=[[1, 128]], channel_multiplier=-1,
        )

        qt = [None] * B
        kt = [None] * B
        vaug = [None] * B
        stage = [None] * B

        # ---- batch 0 QK projection, contraction-outer (DMA-pipelined) ----
        # Uses 8 PSUM banks transiently; the pool closes before the
        # attention-phase PSUM pools open.
        qt[0] = qkp.tile([128, S], bf16, name="qt0", tag="qt")
        kt[0] = qkp.tile([128, S], bf16, name="kt0", tag="kt")
        with tc.tile_pool(name="qk8", bufs=1, space="PSUM") as qk8:
            pq = qk8.tile([128, S], f32, name="pq0", tag="pq")
            pk = qk8.tile([128, S], f32, name="pk0", tag="pk")
            for c in range(NCH):
                st, sp = c == 0, c == NCH - 1
                for q4 in range(S // 512):
                    sl = slice(q4 * 512, (q4 + 1) * 512)
                    nc.tensor.matmul(pq[:, sl], wqT_sb[:, c, :],
                                     xb[0][c][:, sl], start=st, stop=sp)
                    nc.tensor.matmul(pk[:, sl], wkT_sb[:, c, :],
                                     xb[0][c][:, sl], start=st, stop=sp)
            for q4 in range(S // 512):
                sl = slice(q4 * 512, (q4 + 1) * 512)
                nc.vector.tensor_scalar_add(qt[0][:, sl], pq[:, sl], bq_col)
                nc.vector.tensor_scalar_add(kt[0][:, sl], pk[:, sl], bk_col)

        # ---- attention-phase PSUM pools (exactly 8 banks) ----
        sc0 = ctx.enter_context(tc.tile_pool(name="sc0", bufs=1, space="PSUM"))
        sc1 = ctx.enter_context(tc.tile_pool(name="sc1", bufs=1, space="PSUM"))
        pvp = ctx.enter_context(tc.tile_pool(name="pvp", bufs=1, space="PSUM"))
        fil = ctx.enter_context(tc.tile_pool(name="fil", bufs=2, space="PSUM"))

        def v_proj_mgen(b):
            """V projection in [keys, dims] layout + bias/pad/ones -> vaug.
            Two thunks per key tile m (4 contraction chunks each); unit = m
            so attention passes can ensure() the tiles they need."""
            vaug[b] = qkp.tile([128, 2, NKT, HD + 1], bf16,
                               name=f"vaug{b}", tag="vaug")
            items = []
            for m in range(NKT):
                hold = [None]

                def goA(m=m, b=b, hold=hold):
                    rsl = slice(m * 128, (m + 1) * 128)
                    hold[0] = fil.tile([128, 512], f32, name="pv", tag="fil")
                    for c in range(4):
                        nc.tensor.matmul(hold[0][:, 0:LD], xb[b][c][:, rsl],
                                         wvT_sb[:, c, :],
                                         start=(c == 0), stop=False)

                def goB(m=m, b=b, hold=hold):
                    rsl = slice(m * 128, (m + 1) * 128)
                    pvt = hold[0]
                    for c in range(4, NCH):
                        nc.tensor.matmul(pvt[:, 0:LD], xb[b][c][:, rsl],
                                         wvT_sb[:, c, :],
                                         start=False, stop=(c == NCH - 1))
                    tv = work.tile([128, LD], f32, name="tv", tag="tv")
                    nc.vector.tensor_add(tv, pvt[:, 0:LD], bv_bc)
                    pcol = padf[:, b * NKT + m:b * NKT + m + 1]
                    for h in range(2):
                        nc.vector.tensor_scalar_mul(
                            vaug[b][:, h, m, 0:HD], tv[:, h * HD:(h + 1) * HD],
                            pcol)
                        nc.vector.tensor_copy(vaug[b][:, h, m, HD:HD + 1], pcol)
                items.append((m, goA))
                items.append((m, goB))
            return _MGen(items)

        def qk_proj_mgen(b):
            """QK projection as filler thunks (two per 512-row quarter per
            q/k; x for batch b must be resident when these run)."""
            qt[b] = qkp.tile([128, S], bf16, name=f"qt{b}", tag="qt")
            kt[b] = qkp.tile([128, S], bf16, name=f"kt{b}", tag="kt")
            items = []
            for q4 in range(S // 512):
                for wsb, bcol, dstl in ((wqT_sb, bq_col, qt),
                                        (wkT_sb, bk_col, kt)):
                    hold = [None]

                    def goA(q4=q4, b=b, wsb=wsb, hold=hold):
                        sl = slice(q4 * 512, (q4 + 1) * 512)
                        hold[0] = fil.tile([128, 512], f32, name="pq",
                                           tag="fil")
                        for c in range(4):
                            nc.tensor.matmul(hold[0], wsb[:, c, :],
                                             xb[b][c][:, sl],
                                             start=(c == 0), stop=False)

                    def goB(q4=q4, b=b, wsb=wsb, bcol=bcol, dstl=dstl,
                            hold=hold):
                        sl = slice(q4 * 512, (q4 + 1) * 512)
                        for c in range(4, NCH):
                            nc.tensor.matmul(hold[0], wsb[:, c, :],
                                             xb[b][c][:, sl],
                                             start=False, stop=(c == NCH - 1))
                        nc.vector.tensor_scalar_add(dstl[b][:, sl], hold[0],
                                                    bcol)
                    items.append((q4, goA))
                    items.append((q4, goB))
            return _MGen(items)

        def outproj_thunks(b, pp, a2a_out):
            """Output projection for one gathered 128-row chunk (the final
            output rows ride the sync DMA queue)."""
            a2a_sb = stg.tile([128, NCORES, 128], bf16, name=f"a2as{b}{pp}",
                              tag="a2as", bufs=4)

            def load(a2a_sb=a2a_sb, a2a_out=a2a_out):
                nc.sync.dma_start(
                    a2a_sb, a2a_out.rearrange("(j p) r -> p j r", p=128))
            yield load
            r0 = b * RB + pp * 128
            for n in range(D // 512):
                hold = [None]

                def goA(n=n, a2a_sb=a2a_sb, hold=hold):
                    hold[0] = fil.tile([128, 512], f32, name="po", tag="fil")
                    for c in range(4):
                        nc.tensor.matmul(
                            hold[0], a2a_sb[:, c, :],
                            woT_sb[:, c, n * 512:(n + 1) * 512],
                            start=(c == 0), stop=False)
                yield goA

                def goB(n=n, a2a_sb=a2a_sb, r0=r0, hold=hold):
                    for c in range(4, NCH):
                        nc.tensor.matmul(
                            hold[0], a2a_sb[:, c, :],
                            woT_sb[:, c, n * 512:(n + 1) * 512],
                            start=False, stop=(c == NCH - 1))
                    ot = work.tile([128, 512], f32, name="ot", tag="ot")
                    nc.vector.tensor_add(ot, hold[0],
                                         bo_bc[:, n * 512:(n + 1) * 512])
                    nc.sync.dma_start(
                        out.ap()[r0:r0 + 128, n * 512:(n + 1) * 512], ot)
                yield goB

        a2a_filler = [None]

        def outproj_half_thunks(b, p, a2a_out):
            """Output projection for one gathered 64-row (half) chunk."""
            a2a_sb = stg.tile([128, NCORES, 64], bf16, name=f"a2ah{b}{p}",
                              tag="a2ah", bufs=2)

            def load(a2a_sb=a2a_sb, a2a_out=a2a_out):
                nc.sync.dma_start(
                    a2a_sb, a2a_out.rearrange("(j p) r -> p j r", p=128))
            yield load
            r0 = b * RB + (p // 2) * 128 + (p % 2) * 64
            for n in range(D // 512):
                def go(n=n, a2a_sb=a2a_sb, r0=r0):
                    pout = fil.tile([128, 512], f32, name="po", tag="fil")
                    for c in range(NCH):
                        nc.tensor.matmul(
                            pout[0:HD, :], a2a_sb[:, c, :],
                            woT_sb[:, c, n * 512:(n + 1) * 512],
                            start=(c == 0), stop=(c == NCH - 1))
                    ot = work.tile([HD, 512], f32, name="oth", tag="oth")
                    nc.vector.tensor_add(ot, pout[0:HD, :],
                                         bo_bc[0:HD, n * 512:(n + 1) * 512])
                    nc.sync.dma_start(
                        out.ap()[r0:r0 + HD, n * 512:(n + 1) * 512], ot)
                yield go

        def issue_a2a_half(b, p, stage_b):
            """AllToAll a single 512-query pass (64 queries per core) so the
            final chunk's latency chain is as short as possible."""
            a2a_in = dpool.tile([NCORES * 128, 64], bf16,
                                name=f"a2aih{b}{p}", tag="a2aih", bufs=2)
            nc.sync.dma_start(
                a2a_in.rearrange("(j p) r -> p j r", p=128),
                stage_b[:, p * PASS:(p + 1) * PASS]
                .rearrange("p (j r) -> p j r", j=NCORES))
            a2a_out = dpool.tile([NCORES * 128, 64], bf16,
                                 name=f"a2aoh{b}{p}", tag="a2aoh", bufs=2)
            nc.gpsimd.collective_compute(
                "AllToAll", ALU.bypass,
                replica_groups=[list(range(NCORES))],
                ins=[a2a_in.opt()], outs=[a2a_out.opt()])
            a2a_filler[0].add(outproj_half_thunks(b, p, a2a_out), delay=14)

        def issue_a2a(b, pp):
            """AllToAll one 1024-query chunk of batch b's stage buffer; its
            output projection becomes filler work.  The (1, 0) chunk's
            projection is reserved (large delay) so it fills the PE while
            the final chunk's AllToAll drains peer skew."""
            a2a_in = dpool.tile([NCORES * 128, 128], bf16,
                                name=f"a2ai{b}{pp}", tag="a2ai", bufs=4)
            nc.sync.dma_start(
                a2a_in.rearrange("(j p) r -> p j r", p=128),
                stage[b][:, pp * 1024:(pp + 1) * 1024]
                .rearrange("p (j r) -> p j r", j=NCORES))
            a2a_out = dpool.tile([NCORES * 128, 128], bf16,
                                 name=f"a2ao{b}{pp}", tag="a2ao", bufs=4)
            nc.gpsimd.collective_compute(
                "AllToAll", ALU.bypass,
                replica_groups=[list(range(NCORES))],
                ins=[a2a_in.opt()], outs=[a2a_out.opt()])
            delay = 44 if (b == 1 and pp == 0) else 14
            a2a_filler[0].add(outproj_thunks(b, pp, a2a_out), delay=delay)

        def attention(b, filler, vgen, fast_tail=False):
            """Attention for batch b, both heads, 512-query passes.  PV is
            pipelined one score-pair behind exp.  Normalization for a
            1024-query chunk is deferred into the next pass so the PE queue
            never waits on the reciprocal chain."""
            stage[b] = stg.tile([128, S], bf16, name=f"stage{b}", tag="stage")
            pvsb = {}            # (h, pp) -> [65, 1024] f32 SBUF copy of PV
            pending = []         # deferred normalization closures

            def norm_half(p):
                """Per-pass DMA-chain normalization + half A2A (used for the
                next-to-last pass of the fast tail)."""
                pp = p // 2
                csl = slice((p % 2) * PASS, (p % 2 + 1) * PASS)
                qsl = slice(p * PASS, (p + 1) * PASS)
                for h in range(2):
                    ps = pvsb[(h, pp)]
                    dr_den = dpool.tile([PASS], f32, name="drdh",
                                        tag="drdh", bufs=2)
                    nc.sync.dma_start(dr_den, ps[HD:HD + 1, csl])
                    denT = work.tile([128, 4], f32, name="denTh", tag="denTh")
                    nc.sync.dma_start(
                        denT, dr_den.rearrange("(p f) -> p f", p=128))
                    rcpT = work.tile([128, 4], bf16, name="rcpTh", tag="rcpTh")
                    with nc.allow_low_precision(reason="softmax denom bf16"):
                        nc.vector.reciprocal(rcpT, denT)
                    dr_rec = dpool.tile([PASS], bf16, name="drrh",
                                        tag="drrh", bufs=2)
                    nc.sync.dma_start(
                        dr_rec.rearrange("(p f) -> p f", p=128), rcpT)
                    bcs = work.tile([HD, PASS], bf16, name="bcsh", tag="bcsh")
                    nc.sync.dma_start(bcs, dr_rec.partition_broadcast(HD))
                    nc.vector.tensor_mul(
                        stage[b][h * HD:(h + 1) * HD, qsl], ps[0:HD, csl], bcs)
                issue_a2a_half(b, p, stage[b])

            def norm_fast(p):
                """Latency-optimal normalization for the very last pass:
                direct DVE reciprocal + ones-matmul broadcast (no DMA round
                trips), then the half A2A."""
                pp = p // 2
                csl = slice((p % 2) * PASS, (p % 2 + 1) * PASS)
                qsl = slice(p * PASS, (p + 1) * PASS)
                for h in range(2):
                    ps = pvsb[(h, pp)]
                    rec = work.tile([1, PASS], bf16, name="recf", tag="recf")
                    with nc.allow_low_precision(reason="softmax denom bf16"):
                        nc.vector.reciprocal(rec, ps[HD:HD + 1, csl])
                    bc = fil.tile([HD, PASS], f32, name="bcf", tag="fil")
                    nc.tensor.matmul(bc, ones_row, rec, start=True, stop=True)
                    nc.vector.tensor_mul(
                        stage[b][h * HD:(h + 1) * HD, qsl], ps[0:HD, csl], bc)
                issue_a2a_half(b, p, stage[b])

            def norm_pp(pp):
                """Normalize one 1024-query chunk of both heads into the
                staging buffer.  The denominator row is round-tripped
                through DRAM so the reciprocal runs at free-size 8 (instead
                of [1, 1024], where DVE reciprocal costs ~6.4 cyc/elem) and
                the reciprocal is broadcast across partitions by the DMA
                reload; then the chunk's AllToAll is issued."""
                qsl = slice(pp * 1024, (pp + 1) * 1024)
                for h in range(2):
                    ps = pvsb[(h, pp)]
                    dr_den = dpool.tile([2 * PASS], f32, name="drden",
                                        tag="drden", bufs=2)
                    nc.sync.dma_start(dr_den, ps[HD:HD + 1, :])
                    denT = work.tile([128, 8], f32, name="denT", tag="denT")
                    nc.sync.dma_start(
                        denT, dr_den.rearrange("(p f) -> p f", p=128))
                    rcpT = work.tile([128, 8], bf16, name="rcpT", tag="rcpT")
                    with nc.allow_low_precision(reason="softmax denom bf16"):
                        nc.vector.reciprocal(rcpT, denT)
                    dr_rec = dpool.tile([2 * PASS], bf16, name="drrec",
                                        tag="drrec", bufs=2)
                    nc.sync.dma_start(
                        dr_rec.rearrange("(p f) -> p f", p=128), rcpT)
                    bcs = work.tile([HD, 2 * PASS], bf16, name="bcs",
                                    tag="bcs")
                    nc.sync.dma_start(bcs, dr_rec.partition_broadcast(HD))
                    nc.vector.tensor_mul(
                        stage[b][h * HD:(h + 1) * HD, qsl], ps[0:HD, :], bcs)
                issue_a2a(b, pp)

            def run_pending():
                while pending:
                    pending.pop(0)()

            for p in range(NPASS):
                vgen.ensure(4 * p + 3)
                q0 = p * PASS
                ks = list(range(4 * p + 4))      # visible key tiles
                pairs = []
                for i in range(0, len(ks), 2):
                    grp = ks[i:i + 2]
                    pairs.append([(k2, min(PASS, q0 + PASS - 128 * k2))
                                  for k2 in grp])
                npair = len(pairs)
                pv0 = pvp.tile([HD + 1, PASS], f32, name="pv0", tag="pv0")
                pv1 = pvp.tile([HD + 1, PASS], f32, name="pv1", tag="pv1")
                etiles = [None] * npair

                def do_pv(pi):
                    ep, widths = etiles[pi]
                    off = 0
                    for j, (k2, w) in enumerate(widths):
                        st = pi == 0 and j == 0
                        sp = pi == npair - 1 and j == len(widths) - 1
                        psl = slice(PASS - w, PASS)
                        nc.tensor.matmul(pv0[:, psl], vaug[b][:, 0, k2, :],
                                         ep[:, off:off + w],
                                         start=st, stop=sp)
                        nc.tensor.matmul(pv1[:, psl], vaug[b][:, 1, k2, :],
                                         ep[:, 1024 + off:1024 + off + w],
                                         start=st, stop=sp)
                        off += w

                for pi, widths in enumerate(pairs):
                    # scores for this pair, both heads (row-tiled MMs of the
                    # two heads run concurrently on the PE array; separate
                    # per-head PSUM slots keep exp/scores pipelined)
                    sp0 = sc0.tile([128, 1024], f32, name="s0", tag="s0")
                    sp1 = sc1.tile([128, 1024], f32, name="s1", tag="s1")
                    off = 0
                    for k2, w in widths:
                        kA = slice(k2 * 128, (k2 + 1) * 128)
                        qA = slice(q0 + PASS - w, q0 + PASS)
                        nc.tensor.matmul(sp0[:, off:off + w],
                                         kt[b][0:HD, kA], qt[b][0:HD, qA],
                                         start=True, stop=True)
                        nc.tensor.matmul(sp1[:, off:off + w],
                                         kt[b][HD:128, kA], qt[b][HD:128, qA],
                                         start=True, stop=True)
                        off += w
                    if pi == min(2, npair - 1):
                        run_pending()
                    filler.pump(3)
                    # exp (one ACT instruction per head per pair)
                    ep = epool.tile([128, 2048], bf16, name="e", tag="e")
                    etiles[pi] = (ep, widths)
                    nc.scalar.activation(ep[:, 0:off], sp0[:, 0:off],
                                         AF.Exp, scale=0.125)
                    nc.scalar.activation(ep[:, 1024:1024 + off],
                                         sp1[:, 0:off], AF.Exp, scale=0.125)
                    # causal mask on diagonal-starting chunks
                    off = 0
                    for k2, w in widths:
                        if 128 * k2 >= q0:
                            for ho in (0, 1024):
                                nc.vector.tensor_mul(
                                    ep[:, ho + off:ho + off + 128],
                                    ep[:, ho + off:ho + off + 128], diagmask)
                        off += w
                    # PV one pair behind (exp of pair pi still in flight)
                    if pi >= 1:
                        do_pv(pi - 1)
                        filler.pump(1)
                do_pv(npair - 1)
                # free the PV accumulator banks promptly: copy to SBUF, then
                # defer normalization into a later instruction stream
                pp = p // 2
                csl = slice((p % 2) * PASS, (p % 2 + 1) * PASS)
                for h, pv in ((0, pv0), (1, pv1)):
                    if (h, pp) not in pvsb:
                        pvsb[(h, pp)] = qkp.tile(
                            [HD + 1, 2 * PASS], f32, name=f"ps{h}", tag=f"ps{h}")
                    nc.vector.tensor_copy(pvsb[(h, pp)][:, csl], pv)
                if fast_tail and p == NPASS - 2:
                    pending.append(lambda p=p: norm_half(p))
                elif fast_tail and p == NPASS - 1:
                    pass
                elif p % 2 == 1:
                    pending.append(lambda pp=pp: norm_pp(pp))
                filler.pump(2)
                if p == NPASS - 1:
                    run_pending()
                    if fast_tail:
                        a2a_filler[0].pump(16)
                        norm_fast(p)
                    else:
                        filler.pump(3)

        # ---------------- schedule ----------------
        # batch 0's V-projection tail and batch 1's projections fill PE
        # gaps during batch-0 attention; output projections (queued by
        # issue_a2a) and batch 1's V tail fill batch-1 attention.
        vgen0 = v_proj_mgen(0)
        vgen1 = v_proj_mgen(1)
        filler0 = _Filler()
        filler1 = _Filler()
        filler0.add(vgen0)
        filler0.add(qk_proj_mgen(1))
        filler1.add(vgen1)
        a2a_filler[0] = filler1
        attention(0, filler0, vgen0, fast_tail=True)
        filler0.drain()
        attention(1, filler1, vgen1, fast_tail=True)
        filler1.drain()

    nc.finalize()
    return nc


def _get_nc():
    with _lock:
        if "nc" not in _cache:
            _cache["nc"] = _build_nc()
        return _cache["nc"]


def _shard_inputs(x, input_ids, Wq, bq, Wk, bk, Wv, bv, Wo, bo):
    import ml_dtypes
    bf16 = ml_dtypes.bfloat16

    x = np.asarray(x, dtype=np.float32)
    xT = np.ascontiguousarray(x.reshape(R, D).T).astype(bf16)
    woT = np.ascontiguousarray(np.asarray(Wo, dtype=np.float32).T).astype(bf16)
    bo_f = np.asarray(bo, dtype=np.float32)
    ids = np.asarray(input_ids).astype(np.int32)
    # ids_r[p, b*NKT + t] = input_ids[b, t*128 + p]
    ids_r = np.ascontiguousarray(ids.reshape(B, NKT, 128).transpose(2, 0, 1)
                                 .reshape(128, B * NKT))
    Wq = np.asarray(Wq, dtype=np.float32)
    Wk = np.asarray(Wk, dtype=np.float32)
    Wv = np.asarray(Wv, dtype=np.float32)
    bq = np.asarray(bq, dtype=np.float32)
    bk = np.asarray(bk, dtype=np.float32)
    bv = np.asarray(bv, dtype=np.float32)

    in_maps = []
    for c in range(NCORES):
        sl = slice(c * LD, (c + 1) * LD)
        in_maps.append({
            "xT": xT,
            "wqT": np.ascontiguousarray(Wq[sl].T).astype(bf16),
            "wkT": np.ascontiguousarray(Wk[sl].T).astype(bf16),
            "wvT": np.ascontiguousarray(Wv[sl].T).astype(bf16),
            "woT": woT,
            "bq": bq[sl].copy(),
            "bk": bk[sl].copy(),
            "bv": bv[sl].copy(),
            "bo": bo_f,
            "ids": ids_r,
        })
    return in_maps


def run(trace=False, **inputs):
    """Run the kernel; returns (output, BassKernelResults)."""
    from concourse.bass_utils import run_bass_kernel_spmd

    nc = _get_nc()
    in_maps = _shard_inputs(**inputs)
    res = run_bass_kernel_spmd(nc, in_maps, core_ids=list(range(NCORES)),
                               trace=trace)
    full = np.empty((B, S, D), dtype=np.float32)
    for c in range(NCORES):
        o = np.asarray(res.results[c]["out"], dtype=np.float32)
        for b in range(B):
            full[b, c * 128:(c + 1) * 128, :] = \
                o[b * RB:b * RB + 128, :]
            # each batch's last 1024 queries travel as two per-pass half
            # A2As: core c holds queries p*512 + c*64 of passes p = 2, 3
            full[b, 1024 + c * 64:1024 + (c + 1) * 64, :] = \
                o[b * RB + 128:b * RB + 192, :]
            full[b, 1536 + c * 64:1536 + (c + 1) * 64, :] = \
                o[b * RB + 192:b * RB + 256, :]
    return full, res


def kernel(**inputs) -> np.ndarray:
    full, _ = run(trace=False, **inputs)
    return full
